# revision 2
# baseline (speedup 1.0000x reference)
"""Transformer block (pre-LN MHA + FFN) Trainium2 Bass kernel, v2.

Data-parallel over 8 cores: core c handles batch b=c//2, sequence half c%2.
LN1 is computed host-side (x is an input, so h=LN1(x) ships as fp8, like the
baseline's host-shipped x8/xq8); the device runs projections, attention,
LN2 and the FFN.  Attention runs fp8 DoubleRow; softmax exp mostly on ACT
with a tunable Schraudolph (DVE+Pool) offload; FFN1 is 3-term fp8, FFN2 is
2-term fp8 (weight residual) plus a relu-residual correction on the first
PHI2R ff-tiles; the LN2-residual identity term is a single bf16 matmul on
fbf.  LN2 stddev uses Exp(-0.5*Ln(var+eps)) so one activation table serves
the whole kernel.  y/LN2 per-dt work is spread into the attention windows;
only the stats+apply run between windows.

Self-contained: hardcodes shapes B=4, S=2048, D=1024, H=16, FF=4096.
"""

import numpy as np
import ml_dtypes

import concourse.bass as bass
import concourse.bacc as bacc
import concourse.tile as tile
from concourse import mybir

F32 = mybir.dt.float32
BF16 = mybir.dt.bfloat16
FP8 = mybir.dt.float8e4
AF = mybir.ActivationFunctionType
OP = mybir.AluOpType
DRM = mybir.MatmulPerfMode.DoubleRow
I16 = mybir.dt.int16

B, S, D, H, FF = 4, 2048, 1024, 16, 4096
HD = D // H          # 64
P = 128
DT = D // P          # 8  d-tiles
DP = DT // 2         # 4  d-tile pairs (DoubleRow)
FT = FF // P         # 32 ff-tiles
KT = S // P          # 16 k-row tiles
KP = KT // 2         # 8  k-tile pairs
SQ = S // 2          # 1024 own q columns per core
NQ = SQ // 512       # 2 q-chunks of 512
NS = S // 512        # 4 s-chunks of 512
EPS = 1e-5
NCORES = 8
WSC = 32.0                        # fp8 projection-weight pre-scale
EXPS = 0.125 / (WSC * WSC)        # exp scale on raw fp8-scaled scores
EXPB = -float(np.log(4.0))        # exp bias: /4 overflow guard (cancels)
# Schraudolph bit-trick exp for the DVE/Pool offload path
SCHA = 184.6650053 * EXPS
SCHB = 16248.25 + 184.6650053 * EXPB
PHI2R = 12           # ff-tiles 0..PHI2R-1 get the FFN2 relu-residual term
PVLAG = 3            # pv_pair(u - PVLAG) consumption lag

# exp offload: per (qc, t) -> list of kt whose exp goes DVE+Pool
EXPMAP = {}
for _t in range(DT):
    EXPMAP[(0, _t)] = [2, 5, 8, 11]
    EXPMAP[(1, _t)] = [2, 5, 8, 11]
POOLSOLO = {}
for _t in range(DT):
    POOLSOLO[(0, _t)] = [6]
    POOLSOLO[(1, _t)] = []

_CACHE = {}
PHASES = []


def _mark(nc, label):
    PHASES.append((label, int(nc.get_next_instruction_name()[2:])))


def _build_nc():
    nc = bacc.Bacc("TRN2", target_bir_lowering=False, debug=False,
                   num_devices=NCORES)

    h8d = nc.dram_tensor("h8d", [P, DT, S], FP8, kind="ExternalInput")
    xqd = nc.dram_tensor("xqd", [P, DT, SQ], BF16, kind="ExternalInput")
    wq = nc.dram_tensor("wq", [P, DT, DT, P], FP8, kind="ExternalInput")
    wk = nc.dram_tensor("wk", [P, DT, DT, P], FP8, kind="ExternalInput")
    wv = nc.dram_tensor("wv", [P, 2, DT, 512], FP8, kind="ExternalInput")
    w1 = nc.dram_tensor("w1", [P, FT, DT, 2, P], FP8, kind="ExternalInput")
    w2a = nc.dram_tensor("w2a", [P, DT, FT, 2, P], FP8, kind="ExternalInput")
    w2id = nc.dram_tensor("w2id", [P, DT, P], BF16, kind="ExternalInput")
    bq = nc.dram_tensor("bq", [P, DT], F32, kind="ExternalInput")
    bk = nc.dram_tensor("bk", [P, DT], F32, kind="ExternalInput")
    b1 = nc.dram_tensor("b1", [P, FT], F32, kind="ExternalInput")
    b2 = nc.dram_tensor("b2", [P, DT], F32, kind="ExternalInput")
    emat = nc.dram_tensor("emat", [2, DT, P], BF16, kind="ExternalInput")
    OUT = nc.dram_tensor("OUT", [P, DT, SQ], F32, kind="ExternalOutput")

    import os
    repeat = int(os.environ.get("BASS_KERNEL_REPEAT", "1"))
    with tile.TileContext(nc) as tc:
        for _ in range(repeat):
            _emit(nc, tc, h8d, xqd, wq, wk, wv, w1, w2a, w2id,
                  bq, bk, b1, b2, emat, OUT)
    nc.compile()
    return nc


def _emit(nc, tc, h8_d, xq_d, wq_d, wk_d, wv_d, w1_d, w2a_d,
          w2id_d, bq_d, bk_d, b1_d, b2_d, emat_d, OUT_d):
    pools = {}
    pool_objs = {}

    def open_pool(name, bufs, space="SBUF"):
        cm = tc.tile_pool(name=name, bufs=bufs, space=space)
        pools[name] = cm
        pool_objs[name] = cm.__enter__()
        return pool_objs[name]

    def close_pool(name):
        pools.pop(name).__exit__(None, None, None)

    # ---- pools (LIFO open/close discipline) ----
    p_const = open_pool("consts", 1)
    p_ps = open_pool("psg", 2, space="PSUM")           # [128,512] general
    p_sc = open_pool("scps", 2, space="PSUM")          # [128,2,512] scores
    p_ap = open_pool("attps", 2, space="PSUM")         # [65,512] attn accum
    p_per = open_pool("persist", 1)                    # qT8/kz/vaug8/xq...

    # ---- constants (tiles only; DMAs are ordered below) ----
    ones_bf = p_const.tile([P, P], BF16, tag="ones")
    nc.vector.memset(ones_bf[:], 1.0)
    eps_t = p_const.tile([P, 1], F32, tag="eps")
    nc.vector.memset(eps_t[:], EPS)
    expb_t = p_const.tile([P, 1], F32, tag="expb")
    nc.vector.memset(expb_t[:], EXPB)
    sb_bq = p_const.tile([P, DT], F32, tag="bq")
    sb_bk = p_const.tile([P, DT], F32, tag="bk")
    sb_b1 = p_const.tile([P, FT], F32, tag="b1")
    sb_b2 = p_const.tile([P, DT], F32, tag="b2")
    sb_emat = p_const.tile([2, DT, P], BF16, tag="emat")
    sb_w2id = p_const.tile([P, DT, P], BF16, tag="w2id")

    # ---- persistent attention tensors ----
    qT8 = p_per.tile([P, DT, 2, 1024], FP8, tag="qT8")
    kz = p_per.tile([P, DT, KT + 1, P], FP8, tag="kz")
    vaug8 = p_per.tile([P, KT, H, HD + 1], FP8, tag="vaug8")
    xq = p_per.tile([P, DT, SQ], BF16, tag="xq")
    rgather_t = {}

    def rgather_for(qc):
        if qc not in rgather_t:
            rgather_t[qc] = p_per.tile([2, DT, 512], BF16, tag="rgather",
                                       name=f"rgather{qc}")
        return rgather_t[qc]

    nc.vector.memset(qT8[:, 0:4, :, 512:1024], 0.0)
    nc.gpsimd.memset(qT8[:, 4:8, :, 512:1024], 0.0)
    nc.gpsimd.memset(kz[:, :, KT, :], 0.0)             # finite guard tile
    nc.gpsimd.memset(vaug8[:, :, :, HD:HD + 1], WSC)   # ones column = 32

    # y/LN2 + FFN activation pools (persist to the end; opened before the
    # closable attention/h8/qkvw pools to keep LIFO order)
    p_y = open_pool("ybfp", 1)
    p_fT = open_pool("fTp", 1)
    p_l2 = open_pool("ln2tmp", 1)
    p_yt = open_pool("ytmp", 2)
    p_ex = open_pool("exu", 4)                         # exp out, per kt-pair
    p_xi = open_pool("xip", 2)                         # schraudolph i16 stage
    p_st = open_pool("stage", 2)                       # attn psum staging
    p_at = open_pool("attn", 1)                        # attn_raw

    f8_t = {qc: p_fT.tile([P, DT, 512], FP8, tag=f"f8{qc}", name=f"f8{qc}")
            for qc in range(NQ)}
    f8r_t = {qc: p_fT.tile([P, DT, 512], FP8, tag=f"f8r{qc}",
                           name=f"f8r{qc}")
             for qc in range(NQ)}
    fbf_t = {qc: p_fT.tile([P, DT, 512], BF16, tag=f"fbf{qc}",
                           name=f"fbf{qc}")
             for qc in range(NQ)}
    ybf = p_y.tile([P, DT, 512], BF16, tag="ybf")

    # =========================================================
    # DMA queue: h8[0] + first weight slabs + kunit biases lead so the
    # first projection unblocks as early as possible.
    # =========================================================
    p_h8 = open_pool("h8p", 1)
    h8 = p_h8.tile([P, DT, S], FP8, tag="h8")
    p_w = open_pool("qkvw", 2)       # wq/wk slabs (double-buffered)
    p_wv = open_pool("wvp", 1)       # wv slabs

    nc.sync.dma_start(h8[:, :, bass.ts(0, 512)], h8_d[:, :, bass.ts(0, 512)])
    wk0 = p_w.tile([P, DT, P], FP8, tag="wk_s")
    nc.sync.dma_start(wk0[:], wk_d[:, 0, :, :])
    pool_objs["_wk"] = wk0
    nc.sync.dma_start(sb_bk[:], bk_d[:, :])
    wq0 = p_w.tile([P, DT, P], FP8, tag="wq_s")
    nc.sync.dma_start(wq0[:], wq_d[:, 0, :, :])
    pool_objs["_wq0"] = wq0
    nc.sync.dma_start(sb_bq[:], bq_d[:, :])
    nc.sync.dma_start(h8[:, :, bass.ts(1, 512)], h8_d[:, :, bass.ts(1, 512)])
    wv0 = p_wv.tile([P, DT, 512], FP8, tag="wv_s")
    nc.sync.dma_start(wv0[:], wv_d[:, 0, :, :])
    pool_objs["_wv"] = wv0
    for sc in range(2, NS):
        nc.sync.dma_start(h8[:, :, bass.ts(sc, 512)],
                          h8_d[:, :, bass.ts(sc, 512)])
    nc.sync.dma_start(sb_b1[:], b1_d[:, :])
    nc.sync.dma_start(sb_b2[:], b2_d[:, :])
    nc.sync.dma_start(sb_emat[:], emat_d[:, :, :])
    nc.sync.dma_start(sb_w2id[:], w2id_d[:, :, :])
    nc.sync.dma_start(xq[:], xq_d[:, :, :])

    # =========================================================
    # projection units (emitted directly or via the A0 filler queue)
    # =========================================================
    def kunit(t, sc):
        def go():
            if sc == 0 and t > 0:
                wk_s = p_w.tile([P, DT, P], FP8, tag="wk_s")
                nc.sync.dma_start(wk_s[:], wk_d[:, t, :, :])
                pool_objs["_wk"] = wk_s
            wk_s = pool_objs["_wk"]
            pk = p_ps.tile([P, 512], F32, tag="psg")
            for j in range(DP):
                nc.tensor.matmul(pk[:], wk_s[:, 2 * j:2 * j + 2, :],
                                 h8[:, 2 * j:2 * j + 2, bass.ts(sc, 512)],
                                 start=(j == 0), stop=(j == DP - 1),
                                 perf_mode=DRM)
            nc.vector.tensor_scalar(
                kz[:, t, 4 * sc:4 * sc + 4, :],
                pk[:].rearrange("p (a b) -> p a b", a=4),
                sb_bk[:, t:t + 1], None, OP.add)
        return go

    def qunit(t, qc):
        def go():
            if t == 0 and qc == 0:
                wq_s = pool_objs["_wq0"]
            else:
                wq_s = p_w.tile([P, DT, P], FP8, tag="wq_s")
                nc.sync.dma_start(wq_s[:], wq_d[:, t, :, :])
            pq = p_ps.tile([P, 512], F32, tag="psg")
            for j in range(DP):
                nc.tensor.matmul(pq[:], wq_s[:, 2 * j:2 * j + 2, :],
                                 h8[:, 2 * j:2 * j + 2, bass.ts(qc, 512)],
                                 start=(j == 0), stop=(j == DP - 1),
                                 perf_mode=DRM)
            nc.vector.tensor_scalar(qT8[:, t, qc, 0:512], pq[:],
                                    sb_bq[:, t:t + 1], None, OP.add)
        return go

    def vunit(g, kt):
        def go():
            if kt == 0 and g == 1:
                wv_s = p_wv.tile([P, DT, 512], FP8, tag="wv_s")
                nc.sync.dma_start(wv_s[:], wv_d[:, g, :, :])
                pool_objs["_wv"] = wv_s
            wv_s = pool_objs["_wv"]
            pv = p_ps.tile([P, 512], F32, tag="psg")
            for j in range(DP):
                nc.tensor.matmul(pv[:], h8[:, 2 * j:2 * j + 2, bass.ts(kt, P)],
                                 wv_s[:, 2 * j:2 * j + 2, :],
                                 start=(j == 0), stop=(j == DP - 1),
                                 perf_mode=DRM)
            # v-bias is folded into xq host-side (softmax weights sum to 1)
            nc.vector.tensor_copy(
                vaug8[:, kt, 8 * g:8 * g + 8, 0:HD],
                pv[:].rearrange("p (h d) -> p h d", d=HD))
        return go

    # =========================================================
    # attention step for one (t, qc)
    # =========================================================
    def attention_step(qc, t, drain, midwork=None):
        offl = set(EXPMAP.get((qc, t), ()))
        psolo = set(POOLSOLO.get((qc, t), ()))
        drain(t, -1)
        aps = [p_ap.tile([HD + 1, 512], F32, tag="attps",
                         name=f"attps_{qc}_{t}_{i}") for i in range(2)]
        exus = {}

        def pv_pair(u):
            exu_t = exus.pop(u)
            for i in range(2):
                nc.tensor.matmul(aps[i][:],
                                 vaug8[:, 2 * u:2 * u + 2, 2 * t + i, :],
                                 exu_t[:, :, i, :],
                                 start=(u == 0), stop=(u == KP - 1),
                                 perf_mode=DRM)

        for u in range(KP):
            exu_t = p_ex.tile([P, 2, 2, 512], FP8, tag="exu")
            exus[u] = exu_t
            for m in range(2):
                kt = 2 * u + m
                scp = p_sc.tile([P, 2, 512], F32, tag="scps")
                for i in range(2):
                    nc.tensor.matmul(
                        scp[:, i, :],
                        kz[64 * i:64 * i + 64, t, kt:kt + 2, :],
                        qT8[64 * i:64 * i + 64, t, qc, :].rearrange(
                            "p (two f) -> p two f", two=2),
                        start=True, stop=True, perf_mode=DRM)
                if kt in psolo:
                    xi = p_xi.tile([P, 2, 512], I16, tag="xi",
                                   name=f"xi{qc}_{t}_{kt}")
                    nc.gpsimd.tensor_scalar(xi[:], scp[:], SCHA, SCHB,
                                            OP.mult, OP.add)
                    nc.gpsimd.tensor_copy(exu_t[:, m, :, :],
                                          xi[:].bitcast(BF16))
                elif kt in offl:
                    xi = p_xi.tile([P, 2, 512], I16, tag="xi",
                                   name=f"xi{qc}_{t}_{kt}")
                    nc.vector.tensor_scalar(xi[:], scp[:], SCHA, SCHB,
                                            OP.mult, OP.add)
                    nc.gpsimd.tensor_copy(exu_t[:, m, :, :],
                                          xi[:].bitcast(BF16))
                else:
                    nc.scalar.activation(exu_t[:, m, :, :], scp[:], AF.Exp,
                                         bias=expb_t[:], scale=EXPS)
            if u >= PVLAG:
                pv_pair(u - PVLAG)
            drain(t, u)
            if u == 3 and midwork is not None:
                midwork()
        for u in range(KP - PVLAG, KP):
            pv_pair(u)
        for i in range(2):
            st = p_st.tile([HD + 1, 512], BF16, tag="stage")
            nc.vector.tensor_copy(st[:], aps[i][:])
            nc.sync.dma_start(attn_raw[64 * i:64 * i + 64, t, :], st[0:HD, :])
            nc.sync.dma_start(rgather_for(qc)[i:i + 1, t, :],
                              st[HD:HD + 1, :])

    attn_raw = p_at.tile([P, DT, 512], BF16, tag="attn_raw")

    # ---- filler queue machinery ----
    def make_drain(queue, cum, startreq):
        state = {"done": 0}

        def drain(t, u):
            if u < 0:
                tgt = startreq[t]
            else:
                lo = cum[t - 1] if t > 0 else 0
                tgt = lo + ((cum[t] - lo) * (u + 1) + KP - 1) // KP
            while state["done"] < tgt and queue:
                queue.pop(0)()
                state["done"] += 1
        return drain

    # =========================================================
    # y / LN2, split: per-dt part (spread into windows) + finish
    # =========================================================
    def y_dt(qc, dt):
        """y[dt] = attn[dt]/denom + x[dt]  (emitted right after step dt)."""
        rpad = p_yt.tile([2, 512], BF16, tag="rpad", name=f"rpad{qc}_{dt}")
        with nc.allow_low_precision(reason="softmax denom bf16"):
            nc.vector.reciprocal(rpad[:], rgather_for(qc)[:, dt, :])
        rb = p_ps.tile([P, 512], F32, tag="psg")
        nc.tensor.matmul(rb[:], sb_emat[:, dt, :], rpad[:],
                         start=True, stop=True)
        t1 = p_l2.tile([P, 512], F32, tag="t1")
        nc.vector.tensor_mul(t1[:], attn_raw[:, dt, :], rb[:])
        nc.vector.tensor_add(ybf[:, dt, :], t1[:],
                             xq[:, dt, bass.ts(qc, 512)])

    def y_fin(qc):
        """LN2 stats + apply -> fbf/f8/f8r."""
        pst_y = p_ap.tile([P, 512], F32, tag="attps", name=f"psty{qc}")
        pst_q = p_ap.tile([P, 512], F32, tag="attps", name=f"pstq{qc}")
        for dt in range(DT):
            ysq = p_l2.tile([P, 512], BF16, tag="ysq", name=f"ysq{qc}_{dt}")
            nc.vector.tensor_mul(ysq[:], ybf[:, dt, :], ybf[:, dt, :])
            nc.tensor.matmul(pst_y[:], ones_bf[:], ybf[:, dt, :],
                             start=(dt == 0), stop=(dt == DT - 1))
            nc.tensor.matmul(pst_q[:], ones_bf[:], ysq[:],
                             start=(dt == 0), stop=(dt == DT - 1))
        mu = p_l2.tile([P, 512], F32, tag="mu")
        nc.vector.tensor_scalar(mu[:], pst_y[:], 1.0 / D, None, OP.mult)
        msq = p_l2.tile([P, 512], F32, tag="msq")
        nc.vector.tensor_scalar(msq[:], pst_q[:], 1.0 / D, None, OP.mult)
        var = p_l2.tile([P, 512], F32, tag="var")
        nc.vector.tensor_mul(var[:], mu[:], mu[:])
        nc.vector.tensor_sub(var[:], msq[:], var[:])
        nc.scalar.activation(msq[:], var[:], AF.Ln, bias=eps_t[:], scale=1.0)
        rstd = p_l2.tile([P, 512], BF16, tag="rstd")
        with nc.allow_low_precision(reason="rstd bf16 matches matmuls"):
            nc.scalar.activation(rstd[:], msq[:], AF.Exp, scale=-0.5)
        nsb = p_l2.tile([P, 512], BF16, tag="nsb")
        nc.vector.tensor_mul(nsb[:], mu[:], rstd[:])
        for j in range(DP):
            jp = slice(2 * j, 2 * j + 2)
            tmpb = p_l2.tile([P, 2, 512], BF16, tag="tmpb",
                             name=f"tmpb{qc}_{j}")
            nc.vector.tensor_tensor(
                tmpb[:], ybf[:, jp, :],
                rstd[:, None, :].to_broadcast((P, 2, 512)), OP.mult)
            nc.vector.tensor_tensor(
                fbf_t[qc][:, jp, :], tmpb[:],
                nsb[:, None, :].to_broadcast((P, 2, 512)), OP.subtract)
            nc.vector.tensor_copy(f8_t[qc][:, jp, :], fbf_t[qc][:, jp, :])
            nc.gpsimd.tensor_tensor(f8r_t[qc][:, jp, :], fbf_t[qc][:, jp, :],
                                    f8_t[qc][:, jp, :], OP.subtract)

    # =========================================================
    # A0: attention(qc=0), hiding the K/V/Q units
    # =========================================================
    # step 0: k(0,*) + v(0,*) just-in-time for scores/pv, then k(1,*)
    q0 = [kunit(0, 1)]
    q0 += [vunit(0, kt) for kt in range(4)]
    q0 += [kunit(0, 2)]
    q0 += [vunit(0, kt) for kt in range(4, 8)]
    q0 += [kunit(0, 3)]
    q0 += [vunit(0, kt) for kt in range(8, 12)]
    q0 += [kunit(1, 0)]
    q0 += [vunit(0, kt) for kt in range(12, 16)]
    q0 += [kunit(1, 1), kunit(1, 2), kunit(1, 3), qunit(1, 0)]  # ..23
    # steps 1..4: next step's k/q + v(1,*) complete by step 4's pv flush
    q0 += [kunit(2, 0), vunit(1, 0), kunit(2, 1), vunit(1, 1),
           kunit(2, 2), vunit(1, 2), kunit(2, 3), vunit(1, 3),
           qunit(2, 0)]                                       # 24..32
    q0 += [kunit(3, 0), vunit(1, 4), kunit(3, 1), vunit(1, 5),
           kunit(3, 2), vunit(1, 6), kunit(3, 3), vunit(1, 7),
           qunit(3, 0)]                                       # 33..41
    q0 += [kunit(4, 0), vunit(1, 8), kunit(4, 1), vunit(1, 9),
           kunit(4, 2), vunit(1, 10), kunit(4, 3), vunit(1, 11),
           qunit(4, 0)]                                       # 42..50
    q0 += [kunit(5, 0), vunit(1, 12), kunit(5, 1), vunit(1, 13),
           kunit(5, 2), vunit(1, 14), kunit(5, 3), vunit(1, 15),
           qunit(5, 0)]                                       # 51..59
    q0 += [kunit(6, sc) for sc in range(NS)] + [qunit(6, 0)]  # 60..64
    q0 += [qunit(0, 1), qunit(1, 1)]                          # 65..66
    q0 += [kunit(7, sc) for sc in range(NS)] + [qunit(7, 0)]  # 67..71
    q0 += [qunit(t, 1) for t in range(2, DT)]                 # 72..77

    _mark(nc, "prologue")
    kunit(0, 0)()
    qunit(0, 0)()

    cum0 = [24, 33, 42, 51, 60, 67, 74, 78]
    sreq0 = [0, 24, 33, 42, 51, 60, 65, 72]
    drain0 = make_drain(q0, cum0, sreq0)
    for t in range(DT):
        _mark(nc, f"A0.t{t}")
        mw = (lambda tt: (lambda: y_dt(0, tt)))(t - 1) if t > 0 else None
        attention_step(0, t, drain0, midwork=mw)
    while q0:
        q0.pop(0)()
    y_dt(0, DT - 1)

    close_pool("wvp")
    close_pool("qkvw")
    close_pool("h8p")

    # =========================================================
    # FFN machinery: generation A pools (qc=0, inside the A1 window) are
    # opened here; generation B pools (qc=1 tail) open after the attention
    # pools close, with deeper buffering.
    # =========================================================
    gen = {}

    def open_ffn_pools(g, w1bufs, w2bufs):
        gen["f1w"] = open_pool(f"w1slab{g}", w1bufs)
        gen["f2w"] = open_pool(f"w2slab{g}", w2bufs)
        p_fr = open_pool(f"relu{g}", 1)
        gen["fo"] = open_pool(f"fout{g}", 2)
        gen["r8"] = p_fr.tile([P, FT, 512], FP8, tag="r8",
                              name=f"r8{g}")
        gen["r8r"] = p_fr.tile([P, PHI2R, 512], FP8, tag="r8r",
                               name=f"r8r{g}")
        gen["names"] = [f"fout{g}", f"relu{g}", f"w2slab{g}", f"w1slab{g}"]

    def close_ffn_pools():
        for nm in gen.pop("names"):
            close_pool(nm)

    open_ffn_pools("A", 2, 2)

    def f1unit(qc, ft):
        # pre-act = 32*(f@W1g2), 3-term fp8: f8@W8 + f8r@W8 + f8@W8r
        def go():
            if ft % 2 == 0:
                w1_g = gen["f1w"].tile([P, 2, DT, 2, P], FP8, tag="w1s")
                nc.sync.dma_start(w1_g[:], w1_d[:, ft:ft + 2, :, :, :])
                pool_objs["_w1g"] = w1_g
            w1_s = pool_objs["_w1g"][:, ft % 2]
            r8 = gen["r8"]
            r8r = gen["r8r"]
            if qc == 1 and ft % 2 == 1:
                pf2 = p_sc.tile([P, 2, 512], F32, tag="scps",
                                name=f"pf2_{qc}_{ft}")
                pf = pf2[:, 0, :]
            else:
                pft = p_ps.tile([P, 512], F32, tag="psg",
                                name=f"pf_{qc}_{ft}")
                pf = pft[:]
            steps = [(0, f8_t[qc]), (1, f8_t[qc]), (0, f8r_t[qc])]
            for si, (r, src_) in enumerate(steps):
                for j in range(DP):
                    nc.tensor.matmul(pf, w1_s[:, 2 * j:2 * j + 2, r, :],
                                     src_[:, 2 * j:2 * j + 2, :],
                                     start=(si == 0 and j == 0),
                                     stop=(si == 2 and j == DP - 1),
                                     perf_mode=DRM)
            if qc == 1:
                nc.scalar.activation(r8[:, ft, :], pf, AF.Relu,
                                     bias=sb_b1[:, ft:ft + 1], scale=1.0)
            else:
                nc.vector.tensor_scalar(r8[:, ft, :], pf,
                                        sb_b1[:, ft:ft + 1], 0.0,
                                        OP.add, OP.max)
            if ft < PHI2R:
                rbf = p_yt.tile([P, 512], BF16, tag="rbf")
                nc.scalar.activation(rbf[:], pf, AF.Relu,
                                     bias=sb_b1[:, ft:ft + 1], scale=1.0)
                eng = nc.vector if qc == 1 else nc.gpsimd
                eng.tensor_tensor(r8r[:, ft, :], rbf[:], r8[:, ft, :],
                                  OP.subtract)
        return go

    def f2unit(qc, mt):
        # out*2048 = r8@W2_8 + r8@W2r_8 + r8r@W2_8[:PHI2R] + w2id@fbf
        def go():
            r8 = gen["r8"]
            r8r = gen["r8r"]
            slab = gen["f2w"].tile([P, FT, 2, P], FP8, tag="w2s")
            nc.sync.dma_start(slab[:, 0:FT // 2], w2a_d[:, mt, 0:FT // 2])
            nc.sync.dma_start(slab[:, FT // 2:FT], w2a_d[:, mt, FT // 2:FT])
            if qc == 1 and mt % 2 == 1:
                po2 = p_sc.tile([P, 2, 512], F32, tag="scps",
                                name=f"po2_{qc}_{mt}")
                po = po2[:, 0, :]
            else:
                pot = p_ps.tile([P, 512], F32, tag="psg",
                                name=f"po_{qc}_{mt}")
                po = pot[:]
            for si, r in enumerate((0, 1)):
                for u in range(FT // 2):
                    nc.tensor.matmul(po, slab[:, 2 * u:2 * u + 2, r, :],
                                     r8[:, 2 * u:2 * u + 2, :],
                                     start=(si == 0 and u == 0), stop=False,
                                     perf_mode=DRM)
            for u in range(PHI2R // 2):
                nc.tensor.matmul(po, slab[:, 2 * u:2 * u + 2, 0, :],
                                 r8r[:, 2 * u:2 * u + 2, :],
                                 start=False, stop=False, perf_mode=DRM)
            nc.tensor.matmul(po, sb_w2id[:, mt, :], fbf_t[qc][:, mt, :],
                             start=False, stop=True)
            ot = gen["fo"].tile([P, 512], F32, tag="ot")
            nc.vector.tensor_scalar(ot[:], po, 1.0 / 2048.0,
                                    sb_b2[:, mt:mt + 1], OP.mult, OP.add)
            nc.sync.dma_start(OUT_d[:, mt, bass.ts(qc, 512)], ot[:])
        return go

    _mark(nc, "y0fin")
    y_fin(0)

    # =========================================================
    # A1: attention(qc=1), hiding the full FFN of qc=0
    # =========================================================
    q1 = [f1unit(0, ft) for ft in range(FT)] + [f2unit(0, mt)
                                               for mt in range(DT)]
    cum1 = [6, 12, 17, 22, 27, 32, 36, 40]
    sreq1 = [0] * DT
    drain1 = make_drain(q1, cum1, sreq1)
    for t in range(DT):
        _mark(nc, f"A1.t{t}")
        mw = (lambda tt: (lambda: y_dt(1, tt)))(t - 1) if t > 0 else None
        attention_step(1, t, drain1, midwork=mw)
    while q1:
        q1.pop(0)()
    y_dt(1, DT - 1)

    close_ffn_pools()
    close_pool("attn")
    close_pool("stage")
    close_pool("xip")
    close_pool("exu")
    open_ffn_pools("B", 4, 3)

    _mark(nc, "y1fin")
    y_fin(1)
    _mark(nc, "ffn1-tail")
    for ft in range(FT):
        f1unit(1, ft)()
    _mark(nc, "ffn2-tail")
    for mt in range(DT):
        f2unit(1, mt)()
    _mark(nc, "end")

    for name in list(pools)[::-1]:
        close_pool(name)


def _prep_shared(inputs):
    """Host-side weight preprocessing (shared across cores)."""
    f32 = np.float32
    g1 = np.asarray(inputs["g1"], f32)
    beta1 = np.asarray(inputs["beta1"], f32)
    g2 = np.asarray(inputs["g2"], f32)
    beta2 = np.asarray(inputs["beta2"], f32)
    Wq = np.asarray(inputs["Wq"], f32)
    Wk = np.asarray(inputs["Wk"], f32)
    Wv = np.asarray(inputs["Wv"], f32)
    W1 = np.asarray(inputs["W1"], f32)
    W2 = np.asarray(inputs["W2"], f32)

    def fold(Wm, bm):
        Wp = Wm * g1[:, None]
        bp = np.asarray(inputs[bm], f32) + beta1 @ Wm
        return Wp, bp

    Wqp, bqp = fold(Wq, "bq")
    Wkp, bkp = fold(Wk, "bk")
    Wvp, bvp = fold(Wv, "bv")
    W1p = W1 * g2[:, None]
    b1p = np.asarray(inputs["b1"], f32) + beta2 @ W1
    b2p = np.asarray(inputs["b2"], f32) + beta2

    bf = ml_dtypes.bfloat16
    f8 = ml_dtypes.float8_e4m3

    def wtile(Wm, ntile, dtype):
        return np.ascontiguousarray(
            Wm.reshape(ntile, P, Wm.shape[1]).transpose(1, 0, 2)).astype(dtype)

    def wtile_split(Wm, ntile, scale):
        """fp8 value + residual of scale*Wm, packed [P, ntile, 2, cols]."""
        ws = (scale * Wm).astype(f32)
        a = np.clip(ws, -240, 240).astype(f8)
        r = np.clip(ws - a.astype(f32), -240, 240).astype(f8)
        at = wtile(a.astype(f32), ntile, f8)
        rt = wtile(r.astype(f32), ntile, f8)
        return np.ascontiguousarray(np.stack([at, rt], axis=2))

    def btile(bv_, ntile):
        return np.ascontiguousarray(bv_.reshape(ntile, P).T).astype(f32)

    E = np.zeros((2, DT, P), f32)
    for t in range(DT):
        for m in range(P):
            E[m // HD, t, m] = 1.0
    E = E.astype(bf)

    w2id = np.zeros((P, DT, P), f32)
    for mt in range(DT):
        for p in range(P):
            w2id[p, mt, p] = 2048.0 * g2[mt * P + p]
    w2id = w2id.astype(bf)

    wq_t = wtile(WSC * Wqp, DT, f8)
    wk_t = wtile(WSC * Wkp, DT, f8)
    wv_t = wtile(WSC * Wvp, DT, f8)

    def reorder_qk(wt):
        w = wt.reshape(P, DT, DT, P)
        return np.ascontiguousarray(w.transpose(0, 2, 1, 3))

    def reorder_v(wt):
        w = wt.reshape(P, DT, 2, 512)
        return np.ascontiguousarray(w.transpose(0, 2, 1, 3))

    w1s = wtile_split(W1p, DT, WSC)
    w1s = np.ascontiguousarray(
        w1s.reshape(P, DT, 2, FT, P).transpose(0, 3, 1, 2, 4))
    w2s = wtile_split(W2, FT, 64.0)
    w2s = np.ascontiguousarray(
        w2s.reshape(P, FT, 2, DT, P).transpose(0, 3, 1, 2, 4))

    return {
        "wq": reorder_qk(wq_t), "wk": reorder_qk(wk_t),
        "wv": reorder_v(wv_t),
        "w1": w1s, "w2a": w2s, "w2id": w2id,
        "bq": btile(WSC * bqp, DT), "bk": btile(WSC * bkp, DT),
        "bvp": bvp,
        "b1": btile(WSC * b1p, FT), "b2": btile(b2p, DT),
        "emat": E,
    }


def _per_core_inputs(inputs, shared):
    x = np.asarray(inputs["x"], np.float64)
    f8 = ml_dtypes.float8_e4m3
    bf = ml_dtypes.bfloat16
    # exact LN1 on host (g1/beta1 are folded into the projection weights)
    mu = x.mean(-1, keepdims=True)
    var = ((x - mu) ** 2).mean(-1, keepdims=True)
    h = ((x - mu) / np.sqrt(var + EPS)).astype(np.float32)
    xf = np.asarray(inputs["x"], np.float32)
    maps = []
    for c in range(NCORES):
        b, hf = c // 2, c % 2
        hTn = h[b].T.reshape(DT, P, S).transpose(1, 0, 2)
        if hf == 1:
            hTn = np.concatenate([hTn[:, :, SQ:], hTn[:, :, :SQ]], axis=2)
        hTn = np.ascontiguousarray(hTn)
        xTn = xf[b].T.reshape(DT, P, S).transpose(1, 0, 2)
        xown = xTn[:, :, hf * SQ:(hf + 1) * SQ]
        # fold the V bias into the residual: attn(v + bv) = attn(v) + bv
        xown = xown + shared["bvp"].reshape(DT, P).transpose(1, 0)[:, :, None]
        m = {k: v for k, v in shared.items() if k != "bvp"}
        m["h8d"] = np.clip(hTn, -240, 240).astype(f8)
        m["xqd"] = np.ascontiguousarray(xown).astype(bf)
        maps.append(m)
    return maps


def _get_sharded():
    """Build (once) the nc + jitted shard_map executable."""
    if "sharded" in _CACHE:
        return _CACHE["sharded"]

    import jax
    from jax.sharding import Mesh, PartitionSpec
    from jax.experimental.shard_map import shard_map
    from concourse import bass2jax
    from concourse import mybir as _mybir

    bass2jax.install_neuronx_cc_hook()
    nc = _build_nc()

    partition_name = (nc.partition_id_tensor.name
                      if nc.partition_id_tensor else None)
    in_names, out_names, out_avals, zero_shapes = [], [], [], []
    for alloc in nc.m.functions[0].allocations:
        if not isinstance(alloc, _mybir.MemoryLocationSet):
            continue
        name = alloc.memorylocations[0].name
        if alloc.kind == "ExternalInput":
            if name != partition_name:
                in_names.append(name)
        elif alloc.kind == "ExternalOutput":
            shape = tuple(alloc.tensor_shape)
            dtype = _mybir.dt.np(alloc.dtype)
            out_names.append(name)
            out_avals.append(jax.core.ShapedArray(shape, dtype))
            zero_shapes.append((shape, dtype))
    n_params = len(in_names)
    all_names = in_names + out_names
    if partition_name is not None:
        all_names = all_names + [partition_name]
    donate = tuple(range(n_params, n_params + len(out_names)))

    def _body(*args):
        operands = list(args)
        if partition_name is not None:
            operands.append(bass2jax.partition_id_tensor())
        outs = bass2jax._bass_exec_p.bind(
            *operands,
            out_avals=tuple(out_avals),
            in_names=tuple(all_names),
            out_names=tuple(out_names),
            lowering_input_output_aliases=(),
            sim_require_finite=True,
            sim_require_nnan=True,
            nc=nc,
        )
        return tuple(outs)

    devices = jax.devices()[:NCORES]
    mesh = Mesh(np.asarray(devices), ("core",))
    nin = n_params + len(out_names)
    sharded = jax.jit(
        shard_map(_body, mesh=mesh,
                  in_specs=(PartitionSpec("core"),) * nin,
                  out_specs=(PartitionSpec("core"),) * len(out_names),
                  check_rep=False),
        donate_argnums=donate, keep_unused=True)

    _CACHE["sharded"] = (nc, sharded, in_names, out_names, out_avals,
                         zero_shapes)
    return _CACHE["sharded"]


def _concat_inputs(in_maps):
    _, _, in_names, _, _, zero_shapes = _get_sharded()
    concat_in = [
        np.concatenate([np.asarray(in_maps[c][n]) for c in range(NCORES)],
                       axis=0)
        for n in in_names
    ]
    concat_zeros = [
        np.zeros((NCORES * s[0], *s[1:]), d) for (s, d) in zero_shapes
    ]
    return concat_in, concat_zeros


def _run(in_maps):
    nc, fn, in_names, out_names, out_avals, zero_shapes = _get_sharded()
    concat_in, concat_zeros = _concat_inputs(in_maps)
    outs = fn(*concat_in, *concat_zeros)
    res = []
    for c in range(NCORES):
        res.append({
            name: np.asarray(outs[i]).reshape(NCORES, *out_avals[i].shape)[c]
            for i, name in enumerate(out_names)
        })
    return res


def kernel(**inputs):
    shared = _prep_shared(inputs)
    in_maps = _per_core_inputs(inputs, shared)
    res = _run(in_maps)
    out = np.empty((B, S, D), np.float32)
    for c in range(NCORES):
        b, hf = c // 2, c % 2
        o = res[c]["OUT"]                       # [P, DT, SQ]
        out[b, hf * SQ:(hf + 1) * SQ, :] = o.transpose(2, 1, 0).reshape(SQ, D)
    return out


# revision 3
# speedup vs baseline: 1.0028x; 1.0028x over previous
"""Transformer block (pre-LN MHA + FFN) Trainium2 Bass kernel, v2.

Data-parallel over 8 cores: core c handles batch b=c//2, sequence half c%2.
LN1 is computed host-side (x is an input, so h=LN1(x) ships as fp8, like the
baseline's host-shipped x8/xq8); the device runs projections, attention,
LN2 and the FFN.  Attention runs fp8 DoubleRow; softmax exp mostly on ACT
with a tunable Schraudolph (DVE+Pool) offload; FFN1 is 3-term fp8, FFN2 is
2-term fp8 (weight residual) plus a relu-residual correction on the first
PHI2R ff-tiles; the LN2-residual identity term is a single bf16 matmul on
fbf.  LN2 stddev uses Exp(-0.5*Ln(var+eps)) so one activation table serves
the whole kernel.  y/LN2 per-dt work is spread into the attention windows;
only the stats+apply run between windows.

Self-contained: hardcodes shapes B=4, S=2048, D=1024, H=16, FF=4096.
"""

import numpy as np
import ml_dtypes

import concourse.bass as bass
import concourse.bacc as bacc
import concourse.tile as tile
from concourse import mybir

F32 = mybir.dt.float32
BF16 = mybir.dt.bfloat16
FP8 = mybir.dt.float8e4
AF = mybir.ActivationFunctionType
OP = mybir.AluOpType
DRM = mybir.MatmulPerfMode.DoubleRow
I16 = mybir.dt.int16

B, S, D, H, FF = 4, 2048, 1024, 16, 4096
HD = D // H          # 64
P = 128
DT = D // P          # 8  d-tiles
DP = DT // 2         # 4  d-tile pairs (DoubleRow)
FT = FF // P         # 32 ff-tiles
KT = S // P          # 16 k-row tiles
KP = KT // 2         # 8  k-tile pairs
SQ = S // 2          # 1024 own q columns per core
NQ = SQ // 512       # 2 q-chunks of 512
NS = S // 512        # 4 s-chunks of 512
EPS = 1e-5
NCORES = 8
WSC = 32.0                        # fp8 projection-weight pre-scale
EXPS = 0.125 / (WSC * WSC)        # exp scale on raw fp8-scaled scores
EXPB = -float(np.log(4.0))        # exp bias: /4 overflow guard (cancels)
# Schraudolph bit-trick exp for the DVE/Pool offload path
SCHA = 184.6650053 * EXPS
SCHB = 16248.25 + 184.6650053 * EXPB
PHI2R = 12           # ff-tiles 0..PHI2R-1 get the FFN2 relu-residual term
PVLAG = 3            # pv_pair(u - PVLAG) consumption lag

# exp offload: per (qc, t) -> list of kt whose exp goes DVE+Pool
EXPMAP = {}
for _t in range(DT):
    EXPMAP[(0, _t)] = [2, 5, 8, 11]
    EXPMAP[(1, _t)] = [2, 5, 8, 11]
POOLSOLO = {}
for _t in range(DT):
    POOLSOLO[(0, _t)] = [6]
    POOLSOLO[(1, _t)] = []

_CACHE = {}
PHASES = []


def _mark(nc, label):
    PHASES.append((label, int(nc.get_next_instruction_name()[2:])))


def _build_nc():
    nc = bacc.Bacc("TRN2", target_bir_lowering=False, debug=False,
                   num_devices=NCORES)

    h8d = nc.dram_tensor("h8d", [P, DT, S], FP8, kind="ExternalInput")
    xqd = nc.dram_tensor("xqd", [P, DT, SQ], BF16, kind="ExternalInput")
    wq = nc.dram_tensor("wq", [P, DT, DT, P], FP8, kind="ExternalInput")
    wk = nc.dram_tensor("wk", [P, DT, DT, P], FP8, kind="ExternalInput")
    wv = nc.dram_tensor("wv", [P, 2, DT, 512], FP8, kind="ExternalInput")
    w1 = nc.dram_tensor("w1", [P, FT, DT, 2, P], FP8, kind="ExternalInput")
    w2a = nc.dram_tensor("w2a", [P, DT, FT, 2, P], FP8, kind="ExternalInput")
    w2id = nc.dram_tensor("w2id", [P, DT, P], BF16, kind="ExternalInput")
    bq = nc.dram_tensor("bq", [P, DT], F32, kind="ExternalInput")
    bk = nc.dram_tensor("bk", [P, DT], F32, kind="ExternalInput")
    b1 = nc.dram_tensor("b1", [P, FT], F32, kind="ExternalInput")
    b2 = nc.dram_tensor("b2", [P, DT], F32, kind="ExternalInput")
    emat = nc.dram_tensor("emat", [2, DT, P], BF16, kind="ExternalInput")
    OUT = nc.dram_tensor("OUT", [P, DT, SQ], F32, kind="ExternalOutput")

    import os
    repeat = int(os.environ.get("BASS_KERNEL_REPEAT", "1"))
    with tile.TileContext(nc) as tc:
        for _ in range(repeat):
            _emit(nc, tc, h8d, xqd, wq, wk, wv, w1, w2a, w2id,
                  bq, bk, b1, b2, emat, OUT)
    nc.compile()
    return nc


def _emit(nc, tc, h8_d, xq_d, wq_d, wk_d, wv_d, w1_d, w2a_d,
          w2id_d, bq_d, bk_d, b1_d, b2_d, emat_d, OUT_d):
    pools = {}
    pool_objs = {}

    def open_pool(name, bufs, space="SBUF"):
        cm = tc.tile_pool(name=name, bufs=bufs, space=space)
        pools[name] = cm
        pool_objs[name] = cm.__enter__()
        return pool_objs[name]

    def close_pool(name):
        pools.pop(name).__exit__(None, None, None)

    # ---- pools (LIFO open/close discipline) ----
    p_const = open_pool("consts", 1)
    p_ps = open_pool("psg", 2, space="PSUM")           # [128,512] general
    p_sc = open_pool("scps", 2, space="PSUM")          # [128,2,512] scores
    p_ap = open_pool("attps", 2, space="PSUM")         # [65,512] attn accum
    p_per = open_pool("persist", 1)                    # qT8/kz/vaug8/xq...

    # ---- constants (tiles only; DMAs are ordered below) ----
    ones_bf = p_const.tile([P, P], BF16, tag="ones")
    nc.vector.memset(ones_bf[:], 1.0)
    eps_t = p_const.tile([P, 1], F32, tag="eps")
    nc.vector.memset(eps_t[:], EPS)
    expb_t = p_const.tile([P, 1], F32, tag="expb")
    nc.vector.memset(expb_t[:], EXPB)
    sb_bq = p_const.tile([P, DT], F32, tag="bq")
    sb_bk = p_const.tile([P, DT], F32, tag="bk")
    sb_b1 = p_const.tile([P, FT], F32, tag="b1")
    sb_b2 = p_const.tile([P, DT], F32, tag="b2")
    sb_emat = p_const.tile([2, DT, P], BF16, tag="emat")
    sb_w2id = p_const.tile([P, DT, P], BF16, tag="w2id")

    # ---- persistent attention tensors ----
    qT8 = p_per.tile([P, DT, 2, 1024], FP8, tag="qT8")
    kz = p_per.tile([P, DT, KT + 1, P], FP8, tag="kz")
    vaug8 = p_per.tile([P, KT, H, HD + 1], FP8, tag="vaug8")
    xq = p_per.tile([P, DT, SQ], BF16, tag="xq")
    rgather_t = {}

    def rgather_for(qc):
        if qc not in rgather_t:
            rgather_t[qc] = p_per.tile([2, DT, 512], BF16, tag="rgather",
                                       name=f"rgather{qc}")
        return rgather_t[qc]

    nc.vector.memset(qT8[:, 0:4, :, 512:1024], 0.0)
    nc.gpsimd.memset(qT8[:, 4:8, :, 512:1024], 0.0)
    nc.gpsimd.memset(kz[:, :, KT, :], 0.0)             # finite guard tile
    nc.gpsimd.memset(vaug8[:, :, :, HD:HD + 1], WSC)   # ones column = 32

    # y/LN2 + FFN activation pools (persist to the end; opened before the
    # closable attention/h8/qkvw pools to keep LIFO order)
    p_y = open_pool("ybfp", 1)
    p_fT = open_pool("fTp", 1)
    p_l2 = open_pool("ln2tmp", 1)
    p_yt = open_pool("ytmp", 2)
    p_ex = open_pool("exu", 4)                         # exp out, per kt-pair
    p_xi = open_pool("xip", 2)                         # schraudolph i16 stage
    p_st = open_pool("stage", 2)                       # attn psum staging
    p_at = open_pool("attn", 1)                        # attn_raw

    f8_t = {qc: p_fT.tile([P, DT, 512], FP8, tag=f"f8{qc}", name=f"f8{qc}")
            for qc in range(NQ)}
    f8r_t = {qc: p_fT.tile([P, DT, 512], FP8, tag=f"f8r{qc}",
                           name=f"f8r{qc}")
             for qc in range(NQ)}
    fbf_t = {qc: p_fT.tile([P, DT, 512], BF16, tag=f"fbf{qc}",
                           name=f"fbf{qc}")
             for qc in range(NQ)}
    ybf = p_y.tile([P, DT, 512], BF16, tag="ybf")

    # =========================================================
    # DMA queue: h8[0] + first weight slabs + kunit biases lead so the
    # first projection unblocks as early as possible.
    # =========================================================
    p_h8 = open_pool("h8p", 1)
    h8 = p_h8.tile([P, DT, S], FP8, tag="h8")
    p_w = open_pool("qkvw", 2)       # wq/wk slabs (double-buffered)
    p_wv = open_pool("wvp", 1)       # wv slabs

    nc.sync.dma_start(h8[:, :, bass.ts(0, 512)], h8_d[:, :, bass.ts(0, 512)])
    wk0 = p_w.tile([P, DT, P], FP8, tag="wk_s")
    nc.sync.dma_start(wk0[:], wk_d[:, 0, :, :])
    pool_objs["_wk"] = wk0
    nc.sync.dma_start(sb_bk[:], bk_d[:, :])
    wq0 = p_w.tile([P, DT, P], FP8, tag="wq_s")
    nc.sync.dma_start(wq0[:], wq_d[:, 0, :, :])
    pool_objs["_wq0"] = wq0
    nc.sync.dma_start(sb_bq[:], bq_d[:, :])
    nc.sync.dma_start(h8[:, :, bass.ts(1, 512)], h8_d[:, :, bass.ts(1, 512)])
    wv0 = p_wv.tile([P, DT, 512], FP8, tag="wv_s")
    nc.sync.dma_start(wv0[:], wv_d[:, 0, :, :])
    pool_objs["_wv"] = wv0
    for sc in range(2, NS):
        nc.sync.dma_start(h8[:, :, bass.ts(sc, 512)],
                          h8_d[:, :, bass.ts(sc, 512)])
    nc.sync.dma_start(sb_b1[:], b1_d[:, :])
    nc.sync.dma_start(sb_b2[:], b2_d[:, :])
    nc.sync.dma_start(sb_emat[:], emat_d[:, :, :])
    nc.sync.dma_start(sb_w2id[:], w2id_d[:, :, :])
    nc.sync.dma_start(xq[:], xq_d[:, :, :])

    # =========================================================
    # projection units (emitted directly or via the A0 filler queue)
    # =========================================================
    def kunit(t, sc):
        def go():
            if sc == 0 and t > 0:
                wk_s = p_w.tile([P, DT, P], FP8, tag="wk_s")
                nc.sync.dma_start(wk_s[:], wk_d[:, t, :, :])
                pool_objs["_wk"] = wk_s
            wk_s = pool_objs["_wk"]
            pk = p_ps.tile([P, 512], F32, tag="psg")
            for j in range(DP):
                nc.tensor.matmul(pk[:], wk_s[:, 2 * j:2 * j + 2, :],
                                 h8[:, 2 * j:2 * j + 2, bass.ts(sc, 512)],
                                 start=(j == 0), stop=(j == DP - 1),
                                 perf_mode=DRM)
            nc.vector.tensor_scalar(
                kz[:, t, 4 * sc:4 * sc + 4, :],
                pk[:].rearrange("p (a b) -> p a b", a=4),
                sb_bk[:, t:t + 1], None, OP.add)
        return go

    def qunit(t, qc):
        def go():
            if t == 0 and qc == 0:
                wq_s = pool_objs["_wq0"]
            else:
                wq_s = p_w.tile([P, DT, P], FP8, tag="wq_s")
                nc.sync.dma_start(wq_s[:], wq_d[:, t, :, :])
            pq = p_ps.tile([P, 512], F32, tag="psg")
            for j in range(DP):
                nc.tensor.matmul(pq[:], wq_s[:, 2 * j:2 * j + 2, :],
                                 h8[:, 2 * j:2 * j + 2, bass.ts(qc, 512)],
                                 start=(j == 0), stop=(j == DP - 1),
                                 perf_mode=DRM)
            nc.vector.tensor_scalar(qT8[:, t, qc, 0:512], pq[:],
                                    sb_bq[:, t:t + 1], None, OP.add)
        return go

    def vunit(g, kt):
        def go():
            if kt == 0 and g == 1:
                wv_s = p_wv.tile([P, DT, 512], FP8, tag="wv_s")
                nc.sync.dma_start(wv_s[:], wv_d[:, g, :, :])
                pool_objs["_wv"] = wv_s
            wv_s = pool_objs["_wv"]
            pv = p_ps.tile([P, 512], F32, tag="psg")
            for j in range(DP):
                nc.tensor.matmul(pv[:], h8[:, 2 * j:2 * j + 2, bass.ts(kt, P)],
                                 wv_s[:, 2 * j:2 * j + 2, :],
                                 start=(j == 0), stop=(j == DP - 1),
                                 perf_mode=DRM)
            # v-bias is folded into xq host-side (softmax weights sum to 1)
            nc.vector.tensor_copy(
                vaug8[:, kt, 8 * g:8 * g + 8, 0:HD],
                pv[:].rearrange("p (h d) -> p h d", d=HD))
        return go

    # =========================================================
    # attention step for one (t, qc)
    # =========================================================
    def attention_step(qc, t, drain, midwork=None):
        offl = set(EXPMAP.get((qc, t), ()))
        psolo = set(POOLSOLO.get((qc, t), ()))
        drain(t, -1)
        aps = [p_ap.tile([HD + 1, 512], F32, tag="attps",
                         name=f"attps_{qc}_{t}_{i}") for i in range(2)]
        exus = {}

        def pv_pair(u):
            exu_t = exus.pop(u)
            for i in range(2):
                nc.tensor.matmul(aps[i][:],
                                 vaug8[:, 2 * u:2 * u + 2, 2 * t + i, :],
                                 exu_t[:, :, i, :],
                                 start=(u == 0), stop=(u == KP - 1),
                                 perf_mode=DRM)

        for u in range(KP):
            exu_t = p_ex.tile([P, 2, 2, 512], FP8, tag="exu")
            exus[u] = exu_t
            for m in range(2):
                kt = 2 * u + m
                scp = p_sc.tile([P, 2, 512], F32, tag="scps")
                for i in range(2):
                    nc.tensor.matmul(
                        scp[:, i, :],
                        kz[64 * i:64 * i + 64, t, kt:kt + 2, :],
                        qT8[64 * i:64 * i + 64, t, qc, :].rearrange(
                            "p (two f) -> p two f", two=2),
                        start=True, stop=True, perf_mode=DRM)
                if kt in psolo:
                    xi = p_xi.tile([P, 2, 512], I16, tag="xi",
                                   name=f"xi{qc}_{t}_{kt}")
                    nc.gpsimd.tensor_scalar(xi[:], scp[:], SCHA, SCHB,
                                            OP.mult, OP.add)
                    nc.gpsimd.tensor_copy(exu_t[:, m, :, :],
                                          xi[:].bitcast(BF16))
                elif kt in offl:
                    xi = p_xi.tile([P, 2, 512], I16, tag="xi",
                                   name=f"xi{qc}_{t}_{kt}")
                    nc.vector.tensor_scalar(xi[:], scp[:], SCHA, SCHB,
                                            OP.mult, OP.add)
                    nc.gpsimd.tensor_copy(exu_t[:, m, :, :],
                                          xi[:].bitcast(BF16))
                else:
                    nc.scalar.activation(exu_t[:, m, :, :], scp[:], AF.Exp,
                                         bias=expb_t[:], scale=EXPS)
            if u >= PVLAG:
                pv_pair(u - PVLAG)
            drain(t, u)
            if u == 3 and midwork is not None:
                midwork()
        for u in range(KP - PVLAG, KP):
            pv_pair(u)
        for i in range(2):
            st = p_st.tile([HD + 1, 512], BF16, tag="stage")
            nc.vector.tensor_copy(st[:], aps[i][:])
            nc.sync.dma_start(attn_raw[64 * i:64 * i + 64, t, :], st[0:HD, :])
            nc.sync.dma_start(rgather_for(qc)[i:i + 1, t, :],
                              st[HD:HD + 1, :])

    attn_raw = p_at.tile([P, DT, 512], BF16, tag="attn_raw")

    # ---- filler queue machinery ----
    def make_drain(queue, cum, startreq):
        state = {"done": 0}

        def drain(t, u):
            if u < 0:
                tgt = startreq[t]
            else:
                lo = cum[t - 1] if t > 0 else 0
                tgt = lo + ((cum[t] - lo) * (u + 1) + KP - 1) // KP
            while state["done"] < tgt and queue:
                queue.pop(0)()
                state["done"] += 1
        return drain

    # =========================================================
    # y / LN2, split: per-dt part (spread into windows) + finish
    # =========================================================
    def y_dt(qc, dt):
        """y[dt] = attn[dt]/denom + x[dt]  (emitted right after step dt)."""
        rpad = p_yt.tile([2, 512], BF16, tag="rpad", name=f"rpad{qc}_{dt}")
        with nc.allow_low_precision(reason="softmax denom bf16"):
            nc.vector.reciprocal(rpad[:], rgather_for(qc)[:, dt, :])
        rb = p_ps.tile([P, 512], F32, tag="psg")
        nc.tensor.matmul(rb[:], sb_emat[:, dt, :], rpad[:],
                         start=True, stop=True)
        t1 = p_l2.tile([P, 512], F32, tag="t1")
        nc.vector.tensor_mul(t1[:], attn_raw[:, dt, :], rb[:])
        nc.vector.tensor_add(ybf[:, dt, :], t1[:],
                             xq[:, dt, bass.ts(qc, 512)])

    def y_fin(qc):
        """LN2 stats + apply -> fbf/f8/f8r."""
        pool = p_ps if qc == 0 else p_ap
        tag = "psg" if qc == 0 else "attps"
        pst_y = pool.tile([P, 512], F32, tag=tag, name=f"psty{qc}")
        pst_q = pool.tile([P, 512], F32, tag=tag, name=f"pstq{qc}")
        for dt in range(DT):
            ysq = p_l2.tile([P, 512], BF16, tag="ysq", name=f"ysq{qc}_{dt}")
            nc.vector.tensor_mul(ysq[:], ybf[:, dt, :], ybf[:, dt, :])
            nc.tensor.matmul(pst_y[:], ones_bf[:], ybf[:, dt, :],
                             start=(dt == 0), stop=(dt == DT - 1))
            nc.tensor.matmul(pst_q[:], ones_bf[:], ysq[:],
                             start=(dt == 0), stop=(dt == DT - 1))
        mu = p_l2.tile([P, 512], F32, tag="mu")
        nc.vector.tensor_scalar(mu[:], pst_y[:], 1.0 / D, None, OP.mult)
        msq = p_l2.tile([P, 512], F32, tag="msq")
        nc.vector.tensor_scalar(msq[:], pst_q[:], 1.0 / D, None, OP.mult)
        var = p_l2.tile([P, 512], F32, tag="var")
        nc.vector.tensor_mul(var[:], mu[:], mu[:])
        nc.vector.tensor_sub(var[:], msq[:], var[:])
        nc.scalar.activation(msq[:], var[:], AF.Ln, bias=eps_t[:], scale=1.0)
        rstd = p_l2.tile([P, 512], BF16, tag="rstd")
        with nc.allow_low_precision(reason="rstd bf16 matches matmuls"):
            nc.scalar.activation(rstd[:], msq[:], AF.Exp, scale=-0.5)
        nsb = p_l2.tile([P, 512], BF16, tag="nsb")
        nc.vector.tensor_mul(nsb[:], mu[:], rstd[:])
        for j in range(DP):
            jp = slice(2 * j, 2 * j + 2)
            tmpb = p_l2.tile([P, 2, 512], BF16, tag="tmpb",
                             name=f"tmpb{qc}_{j}")
            nc.vector.tensor_tensor(
                tmpb[:], ybf[:, jp, :],
                rstd[:, None, :].to_broadcast((P, 2, 512)), OP.mult)
            nc.vector.tensor_tensor(
                fbf_t[qc][:, jp, :], tmpb[:],
                nsb[:, None, :].to_broadcast((P, 2, 512)), OP.subtract)
            nc.vector.tensor_copy(f8_t[qc][:, jp, :], fbf_t[qc][:, jp, :])
            nc.gpsimd.tensor_tensor(f8r_t[qc][:, jp, :], fbf_t[qc][:, jp, :],
                                    f8_t[qc][:, jp, :], OP.subtract)

    # =========================================================
    # A0: attention(qc=0), hiding the K/V/Q units
    # =========================================================
    # step 0: k(0,*) + v(0,*) just-in-time for scores/pv, then k(1,*)
    q0 = [kunit(0, 1)]
    q0 += [vunit(0, kt) for kt in range(4)]
    q0 += [kunit(0, 2)]
    q0 += [vunit(0, kt) for kt in range(4, 8)]
    q0 += [kunit(0, 3)]
    q0 += [vunit(0, kt) for kt in range(8, 12)]
    q0 += [kunit(1, 0)]
    q0 += [vunit(0, kt) for kt in range(12, 16)]
    q0 += [kunit(1, 1), kunit(1, 2), kunit(1, 3), qunit(1, 0)]  # ..23
    # steps 1..4: next step's k/q + v(1,*) complete by step 4's pv flush
    q0 += [kunit(2, 0), vunit(1, 0), kunit(2, 1), vunit(1, 1),
           kunit(2, 2), vunit(1, 2), kunit(2, 3), vunit(1, 3),
           qunit(2, 0)]                                       # 24..32
    q0 += [kunit(3, 0), vunit(1, 4), kunit(3, 1), vunit(1, 5),
           kunit(3, 2), vunit(1, 6), kunit(3, 3), vunit(1, 7),
           qunit(3, 0)]                                       # 33..41
    q0 += [kunit(4, 0), vunit(1, 8), kunit(4, 1), vunit(1, 9),
           kunit(4, 2), vunit(1, 10), kunit(4, 3), vunit(1, 11),
           qunit(4, 0)]                                       # 42..50
    q0 += [kunit(5, 0), vunit(1, 12), kunit(5, 1), vunit(1, 13),
           kunit(5, 2), vunit(1, 14), kunit(5, 3), vunit(1, 15),
           qunit(5, 0)]                                       # 51..59
    q0 += [kunit(6, sc) for sc in range(NS)] + [qunit(6, 0)]  # 60..64
    q0 += [qunit(0, 1), qunit(1, 1)]                          # 65..66
    q0 += [kunit(7, sc) for sc in range(NS)] + [qunit(7, 0)]  # 67..71
    q0 += [qunit(t, 1) for t in range(2, DT)]                 # 72..77

    _mark(nc, "prologue")
    kunit(0, 0)()
    qunit(0, 0)()

    cum0 = [24, 33, 42, 51, 60, 67, 74, 78]
    sreq0 = [0, 24, 33, 42, 51, 60, 65, 72]
    drain0 = make_drain(q0, cum0, sreq0)
    for t in range(DT):
        _mark(nc, f"A0.t{t}")
        mw = (lambda tt: (lambda: y_dt(0, tt)))(t - 1) if t > 0 else None
        attention_step(0, t, drain0, midwork=mw)
    while q0:
        q0.pop(0)()
    y_dt(0, DT - 1)

    close_pool("wvp")
    close_pool("qkvw")
    close_pool("h8p")

    # =========================================================
    # FFN machinery: generation A pools (qc=0, inside the A1 window) are
    # opened here; generation B pools (qc=1 tail) open after the attention
    # pools close, with deeper buffering.
    # =========================================================
    gen = {}

    def open_ffn_pools(g, w1bufs, w2bufs):
        gen["f1w"] = open_pool(f"w1slab{g}", w1bufs)
        gen["f2w"] = open_pool(f"w2slab{g}", w2bufs)
        p_fr = open_pool(f"relu{g}", 1)
        gen["fo"] = open_pool(f"fout{g}", 2)
        gen["r8"] = p_fr.tile([P, FT, 512], FP8, tag="r8",
                              name=f"r8{g}")
        gen["r8r"] = p_fr.tile([P, PHI2R, 512], FP8, tag="r8r",
                               name=f"r8r{g}")
        gen["names"] = [f"fout{g}", f"relu{g}", f"w2slab{g}", f"w1slab{g}"]

    def close_ffn_pools():
        for nm in gen.pop("names"):
            close_pool(nm)

    open_ffn_pools("A", 2, 2)

    def f1unit(qc, ft):
        # pre-act = 32*(f@W1g2), 3-term fp8: f8@W8 + f8r@W8 + f8@W8r
        def go():
            if ft % 2 == 0:
                w1_g = gen["f1w"].tile([P, 2, DT, 2, P], FP8, tag="w1s")
                nc.sync.dma_start(w1_g[:], w1_d[:, ft:ft + 2, :, :, :])
                pool_objs["_w1g"] = w1_g
            w1_s = pool_objs["_w1g"][:, ft % 2]
            r8 = gen["r8"]
            r8r = gen["r8r"]
            if qc == 1 and ft % 2 == 1:
                pf2 = p_sc.tile([P, 2, 512], F32, tag="scps",
                                name=f"pf2_{qc}_{ft}")
                pf = pf2[:, 0, :]
            else:
                pft = p_ps.tile([P, 512], F32, tag="psg",
                                name=f"pf_{qc}_{ft}")
                pf = pft[:]
            steps = [(0, f8_t[qc]), (1, f8_t[qc]), (0, f8r_t[qc])]
            for si, (r, src_) in enumerate(steps):
                for j in range(DP):
                    nc.tensor.matmul(pf, w1_s[:, 2 * j:2 * j + 2, r, :],
                                     src_[:, 2 * j:2 * j + 2, :],
                                     start=(si == 0 and j == 0),
                                     stop=(si == 2 and j == DP - 1),
                                     perf_mode=DRM)
            if qc == 1:
                nc.scalar.activation(r8[:, ft, :], pf, AF.Relu,
                                     bias=sb_b1[:, ft:ft + 1], scale=1.0)
            else:
                nc.vector.tensor_scalar(r8[:, ft, :], pf,
                                        sb_b1[:, ft:ft + 1], 0.0,
                                        OP.add, OP.max)
            if ft < PHI2R:
                rbf = p_yt.tile([P, 512], BF16, tag="rbf")
                nc.scalar.activation(rbf[:], pf, AF.Relu,
                                     bias=sb_b1[:, ft:ft + 1], scale=1.0)
                eng = nc.vector if qc == 1 else nc.gpsimd
                eng.tensor_tensor(r8r[:, ft, :], rbf[:], r8[:, ft, :],
                                  OP.subtract)
        return go

    def f2unit(qc, mt):
        # out*2048 = r8@W2_8 + r8@W2r_8 + r8r@W2_8[:PHI2R] + w2id@fbf
        def go():
            r8 = gen["r8"]
            r8r = gen["r8r"]
            slab = gen["f2w"].tile([P, FT, 2, P], FP8, tag="w2s")
            nc.sync.dma_start(slab[:, 0:FT // 2], w2a_d[:, mt, 0:FT // 2])
            nc.sync.dma_start(slab[:, FT // 2:FT], w2a_d[:, mt, FT // 2:FT])
            if qc == 1 and mt % 2 == 1:
                po2 = p_sc.tile([P, 2, 512], F32, tag="scps",
                                name=f"po2_{qc}_{mt}")
                po = po2[:, 0, :]
            else:
                pot = p_ps.tile([P, 512], F32, tag="psg",
                                name=f"po_{qc}_{mt}")
                po = pot[:]
            for si, r in enumerate((0, 1)):
                for u in range(FT // 2):
                    nc.tensor.matmul(po, slab[:, 2 * u:2 * u + 2, r, :],
                                     r8[:, 2 * u:2 * u + 2, :],
                                     start=(si == 0 and u == 0), stop=False,
                                     perf_mode=DRM)
            for u in range(PHI2R // 2):
                nc.tensor.matmul(po, slab[:, 2 * u:2 * u + 2, 0, :],
                                 r8r[:, 2 * u:2 * u + 2, :],
                                 start=False, stop=False, perf_mode=DRM)
            nc.tensor.matmul(po, sb_w2id[:, mt, :], fbf_t[qc][:, mt, :],
                             start=False, stop=True)
            ot = gen["fo"].tile([P, 512], F32, tag="ot")
            nc.vector.tensor_scalar(ot[:], po, 1.0 / 2048.0,
                                    sb_b2[:, mt:mt + 1], OP.mult, OP.add)
            nc.sync.dma_start(OUT_d[:, mt, bass.ts(qc, 512)], ot[:])
        return go

    _mark(nc, "y0fin")
    y_fin(0)

    # =========================================================
    # A1: attention(qc=1), hiding the full FFN of qc=0
    # =========================================================
    q1 = [f1unit(0, ft) for ft in range(FT)] + [f2unit(0, mt)
                                               for mt in range(DT)]
    cum1 = [6, 12, 17, 22, 27, 32, 36, 40]
    sreq1 = [0] * DT
    drain1 = make_drain(q1, cum1, sreq1)
    for t in range(DT):
        _mark(nc, f"A1.t{t}")
        mw = (lambda tt: (lambda: y_dt(1, tt)))(t - 1) if t > 0 else None
        attention_step(1, t, drain1, midwork=mw)
    while q1:
        q1.pop(0)()
    y_dt(1, DT - 1)

    _mark(nc, "y1fin")
    y_fin(1)
    for mt in range(DT - 3, DT):
        f2unit(0, mt)()

    close_ffn_pools()
    close_pool("attn")
    close_pool("stage")
    close_pool("xip")
    close_pool("exu")
    open_ffn_pools("B", 4, 3)

    _mark(nc, "ffn1-tail")
    for ft in range(FT):
        f1unit(1, ft)()
    _mark(nc, "ffn2-tail")
    for mt in range(DT):
        f2unit(1, mt)()
    _mark(nc, "end")

    for name in list(pools)[::-1]:
        close_pool(name)


def _prep_shared(inputs):
    """Host-side weight preprocessing (shared across cores)."""
    f32 = np.float32
    g1 = np.asarray(inputs["g1"], f32)
    beta1 = np.asarray(inputs["beta1"], f32)
    g2 = np.asarray(inputs["g2"], f32)
    beta2 = np.asarray(inputs["beta2"], f32)
    Wq = np.asarray(inputs["Wq"], f32)
    Wk = np.asarray(inputs["Wk"], f32)
    Wv = np.asarray(inputs["Wv"], f32)
    W1 = np.asarray(inputs["W1"], f32)
    W2 = np.asarray(inputs["W2"], f32)

    def fold(Wm, bm):
        Wp = Wm * g1[:, None]
        bp = np.asarray(inputs[bm], f32) + beta1 @ Wm
        return Wp, bp

    Wqp, bqp = fold(Wq, "bq")
    Wkp, bkp = fold(Wk, "bk")
    Wvp, bvp = fold(Wv, "bv")
    W1p = W1 * g2[:, None]
    b1p = np.asarray(inputs["b1"], f32) + beta2 @ W1
    b2p = np.asarray(inputs["b2"], f32) + beta2

    bf = ml_dtypes.bfloat16
    f8 = ml_dtypes.float8_e4m3

    def wtile(Wm, ntile, dtype):
        return np.ascontiguousarray(
            Wm.reshape(ntile, P, Wm.shape[1]).transpose(1, 0, 2)).astype(dtype)

    def wtile_split(Wm, ntile, scale):
        """fp8 value + residual of scale*Wm, packed [P, ntile, 2, cols]."""
        ws = (scale * Wm).astype(f32)
        a = np.clip(ws, -240, 240).astype(f8)
        r = np.clip(ws - a.astype(f32), -240, 240).astype(f8)
        at = wtile(a.astype(f32), ntile, f8)
        rt = wtile(r.astype(f32), ntile, f8)
        return np.ascontiguousarray(np.stack([at, rt], axis=2))

    def btile(bv_, ntile):
        return np.ascontiguousarray(bv_.reshape(ntile, P).T).astype(f32)

    E = np.zeros((2, DT, P), f32)
    for t in range(DT):
        for m in range(P):
            E[m // HD, t, m] = 1.0
    E = E.astype(bf)

    w2id = np.zeros((P, DT, P), f32)
    for mt in range(DT):
        for p in range(P):
            w2id[p, mt, p] = 2048.0 * g2[mt * P + p]
    w2id = w2id.astype(bf)

    wq_t = wtile(WSC * Wqp, DT, f8)
    wk_t = wtile(WSC * Wkp, DT, f8)
    wv_t = wtile(WSC * Wvp, DT, f8)

    def reorder_qk(wt):
        w = wt.reshape(P, DT, DT, P)
        return np.ascontiguousarray(w.transpose(0, 2, 1, 3))

    def reorder_v(wt):
        w = wt.reshape(P, DT, 2, 512)
        return np.ascontiguousarray(w.transpose(0, 2, 1, 3))

    w1s = wtile_split(W1p, DT, WSC)
    w1s = np.ascontiguousarray(
        w1s.reshape(P, DT, 2, FT, P).transpose(0, 3, 1, 2, 4))
    w2s = wtile_split(W2, FT, 64.0)
    w2s = np.ascontiguousarray(
        w2s.reshape(P, FT, 2, DT, P).transpose(0, 3, 1, 2, 4))

    return {
        "wq": reorder_qk(wq_t), "wk": reorder_qk(wk_t),
        "wv": reorder_v(wv_t),
        "w1": w1s, "w2a": w2s, "w2id": w2id,
        "bq": btile(WSC * bqp, DT), "bk": btile(WSC * bkp, DT),
        "bvp": bvp,
        "b1": btile(WSC * b1p, FT), "b2": btile(b2p, DT),
        "emat": E,
    }


def _per_core_inputs(inputs, shared):
    x = np.asarray(inputs["x"], np.float64)
    f8 = ml_dtypes.float8_e4m3
    bf = ml_dtypes.bfloat16
    # exact LN1 on host (g1/beta1 are folded into the projection weights)
    mu = x.mean(-1, keepdims=True)
    var = ((x - mu) ** 2).mean(-1, keepdims=True)
    h = ((x - mu) / np.sqrt(var + EPS)).astype(np.float32)
    xf = np.asarray(inputs["x"], np.float32)
    maps = []
    for c in range(NCORES):
        b, hf = c // 2, c % 2
        hTn = h[b].T.reshape(DT, P, S).transpose(1, 0, 2)
        if hf == 1:
            hTn = np.concatenate([hTn[:, :, SQ:], hTn[:, :, :SQ]], axis=2)
        hTn = np.ascontiguousarray(hTn)
        xTn = xf[b].T.reshape(DT, P, S).transpose(1, 0, 2)
        xown = xTn[:, :, hf * SQ:(hf + 1) * SQ]
        # fold the V bias into the residual: attn(v + bv) = attn(v) + bv
        xown = xown + shared["bvp"].reshape(DT, P).transpose(1, 0)[:, :, None]
        m = {k: v for k, v in shared.items() if k != "bvp"}
        m["h8d"] = np.clip(hTn, -240, 240).astype(f8)
        m["xqd"] = np.ascontiguousarray(xown).astype(bf)
        maps.append(m)
    return maps


def _get_sharded():
    """Build (once) the nc + jitted shard_map executable."""
    if "sharded" in _CACHE:
        return _CACHE["sharded"]

    import jax
    from jax.sharding import Mesh, PartitionSpec
    from jax.experimental.shard_map import shard_map
    from concourse import bass2jax
    from concourse import mybir as _mybir

    bass2jax.install_neuronx_cc_hook()
    nc = _build_nc()

    partition_name = (nc.partition_id_tensor.name
                      if nc.partition_id_tensor else None)
    in_names, out_names, out_avals, zero_shapes = [], [], [], []
    for alloc in nc.m.functions[0].allocations:
        if not isinstance(alloc, _mybir.MemoryLocationSet):
            continue
        name = alloc.memorylocations[0].name
        if alloc.kind == "ExternalInput":
            if name != partition_name:
                in_names.append(name)
        elif alloc.kind == "ExternalOutput":
            shape = tuple(alloc.tensor_shape)
            dtype = _mybir.dt.np(alloc.dtype)
            out_names.append(name)
            out_avals.append(jax.core.ShapedArray(shape, dtype))
            zero_shapes.append((shape, dtype))
    n_params = len(in_names)
    all_names = in_names + out_names
    if partition_name is not None:
        all_names = all_names + [partition_name]
    donate = tuple(range(n_params, n_params + len(out_names)))

    def _body(*args):
        operands = list(args)
        if partition_name is not None:
            operands.append(bass2jax.partition_id_tensor())
        outs = bass2jax._bass_exec_p.bind(
            *operands,
            out_avals=tuple(out_avals),
            in_names=tuple(all_names),
            out_names=tuple(out_names),
            lowering_input_output_aliases=(),
            sim_require_finite=True,
            sim_require_nnan=True,
            nc=nc,
        )
        return tuple(outs)

    devices = jax.devices()[:NCORES]
    mesh = Mesh(np.asarray(devices), ("core",))
    nin = n_params + len(out_names)
    sharded = jax.jit(
        shard_map(_body, mesh=mesh,
                  in_specs=(PartitionSpec("core"),) * nin,
                  out_specs=(PartitionSpec("core"),) * len(out_names),
                  check_rep=False),
        donate_argnums=donate, keep_unused=True)

    _CACHE["sharded"] = (nc, sharded, in_names, out_names, out_avals,
                         zero_shapes)
    return _CACHE["sharded"]


def _concat_inputs(in_maps):
    _, _, in_names, _, _, zero_shapes = _get_sharded()
    concat_in = [
        np.concatenate([np.asarray(in_maps[c][n]) for c in range(NCORES)],
                       axis=0)
        for n in in_names
    ]
    concat_zeros = [
        np.zeros((NCORES * s[0], *s[1:]), d) for (s, d) in zero_shapes
    ]
    return concat_in, concat_zeros


def _run(in_maps):
    nc, fn, in_names, out_names, out_avals, zero_shapes = _get_sharded()
    concat_in, concat_zeros = _concat_inputs(in_maps)
    outs = fn(*concat_in, *concat_zeros)
    res = []
    for c in range(NCORES):
        res.append({
            name: np.asarray(outs[i]).reshape(NCORES, *out_avals[i].shape)[c]
            for i, name in enumerate(out_names)
        })
    return res


def kernel(**inputs):
    shared = _prep_shared(inputs)
    in_maps = _per_core_inputs(inputs, shared)
    res = _run(in_maps)
    out = np.empty((B, S, D), np.float32)
    for c in range(NCORES):
        b, hf = c // 2, c % 2
        o = res[c]["OUT"]                       # [P, DT, SQ]
        out[b, hf * SQ:(hf + 1) * SQ, :] = o.transpose(2, 1, 0).reshape(SQ, D)
    return out


# revision 6
# speedup vs baseline: 1.0133x; 1.0105x over previous
"""Transformer block (pre-LN MHA + FFN) Trainium2 Bass kernel, v2.

Data-parallel over 8 cores: core c handles batch b=c//2, sequence half c%2.
LN1 is computed host-side (x is an input, so h=LN1(x) ships as fp8, like the
baseline's host-shipped x8/xq8); the device runs projections, attention,
LN2 and the FFN.  Attention runs fp8 DoubleRow; softmax exp mostly on ACT
with a tunable Schraudolph (DVE+Pool) offload; FFN1 is 3-term fp8, FFN2 is
2-term fp8 (weight residual) plus a relu-residual correction on the first
PHI2R ff-tiles; the LN2-residual identity term is a single bf16 matmul on
fbf.  LN2 stddev uses Exp(-0.5*Ln(var+eps)) so one activation table serves
the whole kernel.  y/LN2 per-dt work is spread into the attention windows;
only the stats+apply run between windows.

Self-contained: hardcodes shapes B=4, S=2048, D=1024, H=16, FF=4096.
"""

import numpy as np
import ml_dtypes

import concourse.bass as bass
import concourse.bacc as bacc
import concourse.tile as tile
from concourse import mybir

F32 = mybir.dt.float32
BF16 = mybir.dt.bfloat16
FP8 = mybir.dt.float8e4
AF = mybir.ActivationFunctionType
OP = mybir.AluOpType
DRM = mybir.MatmulPerfMode.DoubleRow
I16 = mybir.dt.int16

B, S, D, H, FF = 4, 2048, 1024, 16, 4096
HD = D // H          # 64
P = 128
DT = D // P          # 8  d-tiles
DP = DT // 2         # 4  d-tile pairs (DoubleRow)
FT = FF // P         # 32 ff-tiles
KT = S // P          # 16 k-row tiles
KP = KT // 2         # 8  k-tile pairs
SQ = S // 2          # 1024 own q columns per core
NQ = SQ // 512       # 2 q-chunks of 512
NS = S // 512        # 4 s-chunks of 512
EPS = 1e-5
NCORES = 8
WSC = 32.0                        # fp8 projection-weight pre-scale
EXPS = 0.125 / (WSC * WSC)        # exp scale on raw fp8-scaled scores
EXPB = -float(np.log(4.0))        # exp bias: /4 overflow guard (cancels)
# Schraudolph bit-trick exp for the DVE/Pool offload path
SCHA = 184.6650053 * EXPS
SCHB = 16248.25 + 184.6650053 * EXPB
PHI2R = 12           # ff-tiles 0..PHI2R-1 get the FFN2 relu-residual term
PVLAG = 3            # pv_pair(u - PVLAG) consumption lag

# exp offload: per (qc, t) -> list of kt whose exp goes DVE+Pool
EXPMAP = {}
for _t in range(DT):
    EXPMAP[(0, _t)] = [2, 5, 8, 11]
    EXPMAP[(1, _t)] = [2, 5, 8, 11]
POOLSOLO = {}
for _t in range(DT):
    POOLSOLO[(0, _t)] = [6]
    POOLSOLO[(1, _t)] = []

_CACHE = {}
PHASES = []


def _mark(nc, label):
    PHASES.append((label, int(nc.get_next_instruction_name()[2:])))


def _build_nc():
    nc = bacc.Bacc("TRN2", target_bir_lowering=False, debug=False,
                   num_devices=NCORES)

    h8d = nc.dram_tensor("h8d", [P, DT, S], FP8, kind="ExternalInput")
    xqd = nc.dram_tensor("xqd", [P, DT, SQ], BF16, kind="ExternalInput")
    wq = nc.dram_tensor("wq", [P, DT, DT, P], FP8, kind="ExternalInput")
    wk = nc.dram_tensor("wk", [P, DT, DT, P], FP8, kind="ExternalInput")
    wv = nc.dram_tensor("wv", [P, 2, DT, 512], FP8, kind="ExternalInput")
    w1 = nc.dram_tensor("w1", [P, FT, DT, 2, P], FP8, kind="ExternalInput")
    w2a = nc.dram_tensor("w2a", [P, DT, FT, 2, P], FP8, kind="ExternalInput")
    w2id = nc.dram_tensor("w2id", [P, DT, P], BF16, kind="ExternalInput")
    bq = nc.dram_tensor("bq", [P, DT], F32, kind="ExternalInput")
    bk = nc.dram_tensor("bk", [P, DT], F32, kind="ExternalInput")
    b1 = nc.dram_tensor("b1", [P, FT], F32, kind="ExternalInput")
    b2 = nc.dram_tensor("b2", [P, DT], F32, kind="ExternalInput")
    emat = nc.dram_tensor("emat", [2, DT, P], BF16, kind="ExternalInput")
    OUT = nc.dram_tensor("OUT", [P, DT, SQ], F32, kind="ExternalOutput")

    import os
    repeat = int(os.environ.get("BASS_KERNEL_REPEAT", "1"))
    with tile.TileContext(nc) as tc:
        for _ in range(repeat):
            _emit(nc, tc, h8d, xqd, wq, wk, wv, w1, w2a, w2id,
                  bq, bk, b1, b2, emat, OUT)
    nc.compile()
    return nc


def _emit(nc, tc, h8_d, xq_d, wq_d, wk_d, wv_d, w1_d, w2a_d,
          w2id_d, bq_d, bk_d, b1_d, b2_d, emat_d, OUT_d):
    pools = {}
    pool_objs = {}

    def open_pool(name, bufs, space="SBUF"):
        cm = tc.tile_pool(name=name, bufs=bufs, space=space)
        pools[name] = cm
        pool_objs[name] = cm.__enter__()
        return pool_objs[name]

    def close_pool(name):
        pools.pop(name).__exit__(None, None, None)

    # ---- pools (LIFO open/close discipline) ----
    p_const = open_pool("consts", 1)
    p_ps = open_pool("psg", 2, space="PSUM")           # [128,512] general
    p_sc = open_pool("scps", 2, space="PSUM")          # [128,2,512] scores
    p_ap = open_pool("attps", 2, space="PSUM")         # [65,512] attn accum
    p_per = open_pool("persist", 1)                    # qT8/kz/vaug8/xq...

    # ---- constants (tiles only; DMAs are ordered below) ----
    ones_bf = p_const.tile([P, P], BF16, tag="ones")
    nc.vector.memset(ones_bf[:], 1.0)
    eps_t = p_const.tile([P, 1], F32, tag="eps")
    nc.vector.memset(eps_t[:], EPS)
    expb_t = p_const.tile([P, 1], F32, tag="expb")
    nc.vector.memset(expb_t[:], EXPB)
    sb_bq = p_const.tile([P, DT], F32, tag="bq")
    sb_bk = p_const.tile([P, DT], F32, tag="bk")
    sb_b1 = p_const.tile([P, FT], F32, tag="b1")
    sb_b2 = p_const.tile([P, DT], F32, tag="b2")
    sb_emat = p_const.tile([2, DT, P], BF16, tag="emat")
    sb_w2id = p_const.tile([P, DT, P], BF16, tag="w2id")

    # ---- persistent attention tensors ----
    qT8 = p_per.tile([P, DT, 2, 1024], FP8, tag="qT8")
    kz = p_per.tile([P, DT, KT + 1, P], FP8, tag="kz")
    vaug8 = p_per.tile([P, KT, H, HD + 1], FP8, tag="vaug8")
    xq = p_per.tile([P, DT, SQ], BF16, tag="xq")
    rgather_t = {}

    def rgather_for(qc):
        if qc not in rgather_t:
            rgather_t[qc] = p_per.tile([2, DT, 512], BF16, tag="rgather",
                                       name=f"rgather{qc}")
        return rgather_t[qc]

    # PE p-state warmup: ~4us of dummy matmuls while input DMAs land, so
    # the tensor engine is at full clock when the first projection runs.
    warm = p_ps.tile([P, 512], F32, tag="psg", name="warmup")
    for wi in range(26):
        nc.tensor.matmul(warm[:, 0:128], ones_bf[:], ones_bf[:],
                         start=(wi == 0), stop=(wi == 10 - 1))

    nc.vector.memset(qT8[:, 0:4, :, 512:1024], 0.0)
    nc.gpsimd.memset(qT8[:, 4:8, :, 512:1024], 0.0)
    nc.gpsimd.memset(kz[:, :, KT, :], 0.0)             # finite guard tile
    nc.gpsimd.memset(vaug8[:, :, :, HD:HD + 1], WSC)   # ones column = 32

    # y/LN2 + FFN activation pools (persist to the end; opened before the
    # closable attention/h8/qkvw pools to keep LIFO order)
    p_y = open_pool("ybfp", 1)
    p_fT = open_pool("fTp", 1)
    p_l2 = open_pool("ln2tmp", 1)
    p_yt = open_pool("ytmp", 2)
    p_ex = open_pool("exu", 4)                         # exp out, per kt-pair
    p_xi = open_pool("xip", 2)                         # schraudolph i16 stage
    p_st = open_pool("stage", 2)                       # attn psum staging
    p_at = open_pool("attn", 1)                        # attn_raw

    f8_t = {qc: p_fT.tile([P, DT, 512], FP8, tag=f"f8{qc}", name=f"f8{qc}")
            for qc in range(NQ)}
    f8r_t = {qc: p_fT.tile([P, DT, 512], FP8, tag=f"f8r{qc}",
                           name=f"f8r{qc}")
             for qc in range(NQ)}
    fbf_t = {qc: p_fT.tile([P, DT, 512], BF16, tag=f"fbf{qc}",
                           name=f"fbf{qc}")
             for qc in range(NQ)}
    ybf = p_y.tile([P, DT, 512], BF16, tag="ybf")

    # =========================================================
    # DMA queue: h8[0] + first weight slabs + kunit biases lead so the
    # first projection unblocks as early as possible.
    # =========================================================
    p_h8 = open_pool("h8p", 1)
    h8 = p_h8.tile([P, DT, S], FP8, tag="h8")
    p_w = open_pool("qkvw", 2)       # wq/wk slabs (double-buffered)
    p_wv = open_pool("wvp", 1)       # wv slabs

    nc.sync.dma_start(h8[:, :, bass.ts(0, 512)], h8_d[:, :, bass.ts(0, 512)])
    wk0 = p_w.tile([P, DT, P], FP8, tag="wk_s")
    nc.sync.dma_start(wk0[:], wk_d[:, 0, :, :])
    pool_objs["_wk"] = wk0
    nc.sync.dma_start(sb_bk[:], bk_d[:, :])
    wq0 = p_w.tile([P, DT, P], FP8, tag="wq_s")
    nc.sync.dma_start(wq0[:], wq_d[:, 0, :, :])
    pool_objs["_wq0"] = wq0
    nc.sync.dma_start(sb_bq[:], bq_d[:, :])
    nc.sync.dma_start(h8[:, :, bass.ts(1, 512)], h8_d[:, :, bass.ts(1, 512)])
    wv0 = p_wv.tile([P, DT, 512], FP8, tag="wv_s")
    nc.sync.dma_start(wv0[:], wv_d[:, 0, :, :])
    pool_objs["_wv"] = wv0
    for sc in range(2, NS):
        nc.sync.dma_start(h8[:, :, bass.ts(sc, 512)],
                          h8_d[:, :, bass.ts(sc, 512)])
    nc.sync.dma_start(sb_b1[:], b1_d[:, :])
    nc.sync.dma_start(sb_b2[:], b2_d[:, :])
    nc.sync.dma_start(sb_emat[:], emat_d[:, :, :])
    nc.sync.dma_start(sb_w2id[:], w2id_d[:, :, :])
    nc.sync.dma_start(xq[:], xq_d[:, :, :])

    # =========================================================
    # projection units (emitted directly or via the A0 filler queue)
    # =========================================================
    def kunit(t, sc):
        def go():
            if sc == 0 and t > 0:
                wk_s = p_w.tile([P, DT, P], FP8, tag="wk_s")
                nc.sync.dma_start(wk_s[:], wk_d[:, t, :, :])
                pool_objs["_wk"] = wk_s
            wk_s = pool_objs["_wk"]
            pk = p_ps.tile([P, 512], F32, tag="psg")
            for j in range(DP):
                nc.tensor.matmul(pk[:], wk_s[:, 2 * j:2 * j + 2, :],
                                 h8[:, 2 * j:2 * j + 2, bass.ts(sc, 512)],
                                 start=(j == 0), stop=(j == DP - 1),
                                 perf_mode=DRM)
            nc.scalar.activation(
                kz[:, t, 4 * sc:4 * sc + 4, :],
                pk[:].rearrange("p (a b) -> p a b", a=4),
                AF.Identity, bias=sb_bk[:, t:t + 1], scale=1.0)
        return go

    def qunit(t, qc):
        def go():
            if t == 0 and qc == 0:
                wq_s = pool_objs["_wq0"]
            else:
                wq_s = p_w.tile([P, DT, P], FP8, tag="wq_s")
                nc.sync.dma_start(wq_s[:], wq_d[:, t, :, :])
            pq = p_ps.tile([P, 512], F32, tag="psg")
            for j in range(DP):
                nc.tensor.matmul(pq[:], wq_s[:, 2 * j:2 * j + 2, :],
                                 h8[:, 2 * j:2 * j + 2, bass.ts(qc, 512)],
                                 start=(j == 0), stop=(j == DP - 1),
                                 perf_mode=DRM)
            nc.vector.tensor_scalar(qT8[:, t, qc, 0:512], pq[:],
                                    sb_bq[:, t:t + 1], None, OP.add)
        return go

    def vunit(g, kt):
        def go():
            if kt == 0 and g == 1:
                wv_s = p_wv.tile([P, DT, 512], FP8, tag="wv_s")
                nc.sync.dma_start(wv_s[:], wv_d[:, g, :, :])
                pool_objs["_wv"] = wv_s
            wv_s = pool_objs["_wv"]
            pv = p_ps.tile([P, 512], F32, tag="psg")
            for j in range(DP):
                nc.tensor.matmul(pv[:], h8[:, 2 * j:2 * j + 2, bass.ts(kt, P)],
                                 wv_s[:, 2 * j:2 * j + 2, :],
                                 start=(j == 0), stop=(j == DP - 1),
                                 perf_mode=DRM)
            # v-bias is folded into xq host-side (softmax weights sum to 1)
            if g == 0 and kt % 2 == 1:
                nc.scalar.activation(
                    vaug8[:, kt, 8 * g:8 * g + 8, 0:HD],
                    pv[:].rearrange("p (h d) -> p h d", d=HD),
                    AF.Copy)
            else:
                nc.vector.tensor_copy(
                    vaug8[:, kt, 8 * g:8 * g + 8, 0:HD],
                    pv[:].rearrange("p (h d) -> p h d", d=HD))
        return go

    # =========================================================
    # attention step for one (t, qc)
    # =========================================================
    def attention_step(qc, t, drain, midwork=None):
        offl = set(EXPMAP.get((qc, t), ()))
        psolo = set(POOLSOLO.get((qc, t), ()))
        drain(t, -1)
        aps = [p_ap.tile([HD + 1, 512], F32, tag="attps",
                         name=f"attps_{qc}_{t}_{i}") for i in range(2)]
        exus = {}

        def pv_pair(u):
            exu_t = exus.pop(u)
            for i in range(2):
                nc.tensor.matmul(aps[i][:],
                                 vaug8[:, 2 * u:2 * u + 2, 2 * t + i, :],
                                 exu_t[:, :, i, :],
                                 start=(u == 0), stop=(u == KP - 1),
                                 perf_mode=DRM)

        for u in range(KP):
            exu_t = p_ex.tile([P, 2, 2, 512], FP8, tag="exu")
            exus[u] = exu_t
            for m in range(2):
                kt = 2 * u + m
                scp = p_sc.tile([P, 2, 512], F32, tag="scps")
                for i in range(2):
                    nc.tensor.matmul(
                        scp[:, i, :],
                        kz[64 * i:64 * i + 64, t, kt:kt + 2, :],
                        qT8[64 * i:64 * i + 64, t, qc, :].rearrange(
                            "p (two f) -> p two f", two=2),
                        start=True, stop=True, perf_mode=DRM)
                if kt in psolo:
                    xi = p_xi.tile([P, 2, 512], I16, tag="xi",
                                   name=f"xi{qc}_{t}_{kt}")
                    nc.gpsimd.tensor_scalar(xi[:], scp[:], SCHA, SCHB,
                                            OP.mult, OP.add)
                    nc.gpsimd.tensor_copy(exu_t[:, m, :, :],
                                          xi[:].bitcast(BF16))
                elif kt in offl:
                    xi = p_xi.tile([P, 2, 512], I16, tag="xi",
                                   name=f"xi{qc}_{t}_{kt}")
                    nc.vector.tensor_scalar(xi[:], scp[:], SCHA, SCHB,
                                            OP.mult, OP.add)
                    nc.gpsimd.tensor_copy(exu_t[:, m, :, :],
                                          xi[:].bitcast(BF16))
                else:
                    nc.scalar.activation(exu_t[:, m, :, :], scp[:], AF.Exp,
                                         bias=expb_t[:], scale=EXPS)
            if u >= PVLAG:
                pv_pair(u - PVLAG)
            drain(t, u)
            if u == 3 and midwork is not None:
                midwork()
        for u in range(KP - PVLAG, KP):
            pv_pair(u)
        for i in range(2):
            st = p_st.tile([HD + 1, 512], BF16, tag="stage")
            nc.vector.tensor_copy(st[:], aps[i][:])
            nc.sync.dma_start(attn_raw[64 * i:64 * i + 64, t, :], st[0:HD, :])
            nc.sync.dma_start(rgather_for(qc)[i:i + 1, t, :],
                              st[HD:HD + 1, :])

    attn_raw = p_at.tile([P, DT, 512], BF16, tag="attn_raw")

    # ---- filler queue machinery ----
    def make_drain(queue, cum, startreq):
        state = {"done": 0}

        def drain(t, u):
            if u < 0:
                tgt = startreq[t]
            else:
                lo = cum[t - 1] if t > 0 else 0
                tgt = lo + ((cum[t] - lo) * (u + 1) + KP - 1) // KP
            while state["done"] < tgt and queue:
                queue.pop(0)()
                state["done"] += 1
        return drain

    # =========================================================
    # y / LN2, split: per-dt part (spread into windows) + finish
    # =========================================================
    def y_dt(qc, dt):
        """y[dt] = attn[dt]/denom + x[dt]  (emitted right after step dt)."""
        rpad = p_yt.tile([2, 512], BF16, tag="rpad", name=f"rpad{qc}_{dt}")
        with nc.allow_low_precision(reason="softmax denom bf16"):
            nc.vector.reciprocal(rpad[:], rgather_for(qc)[:, dt, :])
        rb = p_ps.tile([P, 512], F32, tag="psg")
        nc.tensor.matmul(rb[:], sb_emat[:, dt, :], rpad[:],
                         start=True, stop=True)
        t1 = p_l2.tile([P, 512], F32, tag="t1")
        nc.vector.tensor_mul(t1[:], attn_raw[:, dt, :], rb[:])
        nc.gpsimd.tensor_tensor(ybf[:, dt, :], t1[:],
                                xq[:, dt, bass.ts(qc, 512)], OP.add)

    def y_fin(qc):
        """LN2 stats + apply -> fbf/f8/f8r."""
        pool = p_ps if qc == 0 else p_ap
        tag = "psg" if qc == 0 else "attps"
        pst_y = pool.tile([P, 512], F32, tag=tag, name=f"psty{qc}")
        pst_q = pool.tile([P, 512], F32, tag=tag, name=f"pstq{qc}")
        for dt in range(DT):
            ysq = p_l2.tile([P, 512], BF16, tag="ysq", name=f"ysq{qc}_{dt}")
            nc.vector.tensor_mul(ysq[:], ybf[:, dt, :], ybf[:, dt, :])
            nc.tensor.matmul(pst_y[:], ones_bf[:], ybf[:, dt, :],
                             start=(dt == 0), stop=(dt == DT - 1))
            nc.tensor.matmul(pst_q[:], ones_bf[:], ysq[:],
                             start=(dt == 0), stop=(dt == DT - 1))
        mu = p_l2.tile([P, 512], F32, tag="mu")
        nc.vector.tensor_scalar(mu[:], pst_y[:], 1.0 / D, None, OP.mult)
        msq = p_l2.tile([P, 512], F32, tag="msq")
        nc.vector.tensor_scalar(msq[:], pst_q[:], 1.0 / D, None, OP.mult)
        var = p_l2.tile([P, 512], F32, tag="var")
        nc.vector.tensor_mul(var[:], mu[:], mu[:])
        nc.vector.tensor_sub(var[:], msq[:], var[:])
        nc.scalar.activation(msq[:], var[:], AF.Ln, bias=eps_t[:], scale=1.0)
        rstd = p_l2.tile([P, 512], BF16, tag="rstd")
        with nc.allow_low_precision(reason="rstd bf16 matches matmuls"):
            nc.scalar.activation(rstd[:], msq[:], AF.Exp, scale=-0.5)
        nsb = p_l2.tile([P, 512], BF16, tag="nsb")
        nc.vector.tensor_mul(nsb[:], mu[:], rstd[:])
        for j in range(DP):
            jp = slice(2 * j, 2 * j + 2)
            tmpb = p_l2.tile([P, 2, 512], BF16, tag="tmpb",
                             name=f"tmpb{qc}_{j}")
            nc.vector.tensor_tensor(
                tmpb[:], ybf[:, jp, :],
                rstd[:, None, :].to_broadcast((P, 2, 512)), OP.mult)
            nc.vector.tensor_tensor(
                fbf_t[qc][:, jp, :], tmpb[:],
                nsb[:, None, :].to_broadcast((P, 2, 512)), OP.subtract)
            nc.vector.tensor_copy(f8_t[qc][:, jp, :], fbf_t[qc][:, jp, :])
            nc.gpsimd.tensor_tensor(f8r_t[qc][:, jp, :], fbf_t[qc][:, jp, :],
                                    f8_t[qc][:, jp, :], OP.subtract)

    # =========================================================
    # A0: attention(qc=0), hiding the K/V/Q units
    # =========================================================
    # step 0: k(0,*) + v(0,*) just-in-time for scores/pv, then k(1,*)
    q0 = [kunit(0, 1)]
    q0 += [vunit(0, kt) for kt in range(4)]
    q0 += [kunit(0, 2)]
    q0 += [vunit(0, kt) for kt in range(4, 8)]
    q0 += [kunit(0, 3)]
    q0 += [vunit(0, kt) for kt in range(8, 12)]
    q0 += [kunit(1, 0)]
    q0 += [vunit(0, kt) for kt in range(12, 16)]
    q0 += [kunit(1, 1), kunit(1, 2), kunit(1, 3), qunit(1, 0)]  # ..23
    # steps 1..4: next step's k/q + v(1,*) complete by step 4's pv flush
    q0 += [kunit(2, 0), vunit(1, 0), kunit(2, 1), vunit(1, 1),
           kunit(2, 2), vunit(1, 2), kunit(2, 3), vunit(1, 3),
           qunit(2, 0)]                                       # 24..32
    q0 += [kunit(3, 0), vunit(1, 4), kunit(3, 1), vunit(1, 5),
           kunit(3, 2), vunit(1, 6), kunit(3, 3), vunit(1, 7),
           qunit(3, 0)]                                       # 33..41
    q0 += [kunit(4, 0), vunit(1, 8), kunit(4, 1), vunit(1, 9),
           kunit(4, 2), vunit(1, 10), kunit(4, 3), vunit(1, 11),
           qunit(4, 0)]                                       # 42..50
    q0 += [kunit(5, 0), vunit(1, 12), kunit(5, 1), vunit(1, 13),
           kunit(5, 2), vunit(1, 14), kunit(5, 3), vunit(1, 15),
           qunit(5, 0)]                                       # 51..59
    q0 += [kunit(6, sc) for sc in range(NS)] + [qunit(6, 0)]  # 60..64
    q0 += [qunit(0, 1), qunit(1, 1)]                          # 65..66
    q0 += [kunit(7, sc) for sc in range(NS)] + [qunit(7, 0)]  # 67..71
    q0 += [qunit(t, 1) for t in range(2, DT)]                 # 72..77

    _mark(nc, "prologue")
    kunit(0, 0)()
    qunit(0, 0)()

    cum0 = [24, 33, 42, 51, 60, 67, 74, 78]
    sreq0 = [0, 24, 33, 42, 51, 60, 65, 72]
    drain0 = make_drain(q0, cum0, sreq0)
    for t in range(DT):
        _mark(nc, f"A0.t{t}")
        mw = (lambda tt: (lambda: y_dt(0, tt)))(t - 1) if t > 0 else None
        attention_step(0, t, drain0, midwork=mw)
    while q0:
        q0.pop(0)()
    y_dt(0, DT - 1)

    close_pool("wvp")
    close_pool("qkvw")
    close_pool("h8p")

    # =========================================================
    # FFN machinery: generation A pools (qc=0, inside the A1 window) are
    # opened here; generation B pools (qc=1 tail) open after the attention
    # pools close, with deeper buffering.
    # =========================================================
    gen = {}

    def open_ffn_pools(g, w1bufs, w2bufs):
        gen["f1w"] = open_pool(f"w1slab{g}", w1bufs)
        gen["f2w"] = open_pool(f"w2slab{g}", w2bufs)
        p_fr = open_pool(f"relu{g}", 1)
        gen["fo"] = open_pool(f"fout{g}", 2)
        gen["r8"] = p_fr.tile([P, FT, 512], FP8, tag="r8",
                              name=f"r8{g}")
        gen["r8r"] = p_fr.tile([P, PHI2R, 512], FP8, tag="r8r",
                               name=f"r8r{g}")
        gen["names"] = [f"fout{g}", f"relu{g}", f"w2slab{g}", f"w1slab{g}"]

    def close_ffn_pools():
        for nm in gen.pop("names"):
            close_pool(nm)

    open_ffn_pools("A", 2, 2)

    def f1unit(qc, ft):
        # pre-act = 32*(f@W1g2), 3-term fp8: f8@W8 + f8r@W8 + f8@W8r
        def go():
            if ft % 2 == 0:
                w1_g = gen["f1w"].tile([P, 2, DT, 2, P], FP8, tag="w1s")
                nc.sync.dma_start(w1_g[:], w1_d[:, ft:ft + 2, :, :, :])
                pool_objs["_w1g"] = w1_g
            w1_s = pool_objs["_w1g"][:, ft % 2]
            r8 = gen["r8"]
            r8r = gen["r8r"]
            if qc == 1 and ft % 2 == 1:
                pf2 = p_sc.tile([P, 2, 512], F32, tag="scps",
                                name=f"pf2_{qc}_{ft}")
                pf = pf2[:, 0, :]
            else:
                pft = p_ps.tile([P, 512], F32, tag="psg",
                                name=f"pf_{qc}_{ft}")
                pf = pft[:]
            steps = [(0, f8_t[qc]), (1, f8_t[qc]), (0, f8r_t[qc])]
            for si, (r, src_) in enumerate(steps):
                for j in range(DP):
                    nc.tensor.matmul(pf, w1_s[:, 2 * j:2 * j + 2, r, :],
                                     src_[:, 2 * j:2 * j + 2, :],
                                     start=(si == 0 and j == 0),
                                     stop=(si == 2 and j == DP - 1),
                                     perf_mode=DRM)
            if qc == 1:
                nc.scalar.activation(r8[:, ft, :], pf, AF.Relu,
                                     bias=sb_b1[:, ft:ft + 1], scale=1.0)
            else:
                nc.vector.tensor_scalar(r8[:, ft, :], pf,
                                        sb_b1[:, ft:ft + 1], 0.0,
                                        OP.add, OP.max)
            if ft < PHI2R:
                rbf = p_yt.tile([P, 512], BF16, tag="rbf")
                nc.scalar.activation(rbf[:], pf, AF.Relu,
                                     bias=sb_b1[:, ft:ft + 1], scale=1.0)
                eng = nc.vector if qc == 1 else nc.gpsimd
                eng.tensor_tensor(r8r[:, ft, :], rbf[:], r8[:, ft, :],
                                  OP.subtract)
        return go

    def f2unit(qc, mt):
        # out*2048 = r8@W2_8 + r8@W2r_8 + r8r@W2_8[:PHI2R] + w2id@fbf
        def go():
            r8 = gen["r8"]
            r8r = gen["r8r"]
            slab = gen["f2w"].tile([P, FT, 2, P], FP8, tag="w2s")
            nc.sync.dma_start(slab[:, 0:FT // 2], w2a_d[:, mt, 0:FT // 2])
            nc.sync.dma_start(slab[:, FT // 2:FT], w2a_d[:, mt, FT // 2:FT])
            if qc == 1 and mt % 2 == 1:
                po2 = p_sc.tile([P, 2, 512], F32, tag="scps",
                                name=f"po2_{qc}_{mt}")
                po = po2[:, 0, :]
            else:
                pot = p_ps.tile([P, 512], F32, tag="psg",
                                name=f"po_{qc}_{mt}")
                po = pot[:]
            for si, r in enumerate((0, 1)):
                for u in range(FT // 2):
                    nc.tensor.matmul(po, slab[:, 2 * u:2 * u + 2, r, :],
                                     r8[:, 2 * u:2 * u + 2, :],
                                     start=(si == 0 and u == 0), stop=False,
                                     perf_mode=DRM)
            for u in range(PHI2R // 2):
                nc.tensor.matmul(po, slab[:, 2 * u:2 * u + 2, 0, :],
                                 r8r[:, 2 * u:2 * u + 2, :],
                                 start=False, stop=False, perf_mode=DRM)
            nc.tensor.matmul(po, sb_w2id[:, mt, :], fbf_t[qc][:, mt, :],
                             start=False, stop=True)
            ot = gen["fo"].tile([P, 512], F32, tag="ot")
            nc.vector.tensor_scalar(ot[:], po, 1.0 / 2048.0,
                                    sb_b2[:, mt:mt + 1], OP.mult, OP.add)
            nc.sync.dma_start(OUT_d[:, mt, bass.ts(qc, 512)], ot[:])
        return go

    _mark(nc, "y0fin")
    y_fin(0)

    # =========================================================
    # A1: attention(qc=1), hiding the full FFN of qc=0
    # =========================================================
    q1 = [f1unit(0, ft) for ft in range(FT)] + [f2unit(0, mt)
                                               for mt in range(DT)]
    cum1 = [6, 12, 17, 22, 27, 32, 36, 40]
    sreq1 = [0] * DT
    drain1 = make_drain(q1, cum1, sreq1)
    for t in range(DT):
        _mark(nc, f"A1.t{t}")
        mw = (lambda tt: (lambda: y_dt(1, tt)))(t - 1) if t > 0 else None
        attention_step(1, t, drain1, midwork=mw)
    while q1:
        q1.pop(0)()
    y_dt(1, DT - 1)

    _mark(nc, "y1fin")
    y_fin(1)
    for mt in range(DT - 3, DT):
        f2unit(0, mt)()

    close_ffn_pools()
    close_pool("attn")
    close_pool("stage")
    close_pool("xip")
    close_pool("exu")
    open_ffn_pools("B", 4, 3)

    _mark(nc, "ffn1-tail")
    for ft in range(FT):
        f1unit(1, ft)()
    _mark(nc, "ffn2-tail")
    for mt in range(DT):
        f2unit(1, mt)()
    _mark(nc, "end")

    for name in list(pools)[::-1]:
        close_pool(name)


def _prep_shared(inputs):
    """Host-side weight preprocessing (shared across cores)."""
    f32 = np.float32
    g1 = np.asarray(inputs["g1"], f32)
    beta1 = np.asarray(inputs["beta1"], f32)
    g2 = np.asarray(inputs["g2"], f32)
    beta2 = np.asarray(inputs["beta2"], f32)
    Wq = np.asarray(inputs["Wq"], f32)
    Wk = np.asarray(inputs["Wk"], f32)
    Wv = np.asarray(inputs["Wv"], f32)
    W1 = np.asarray(inputs["W1"], f32)
    W2 = np.asarray(inputs["W2"], f32)

    def fold(Wm, bm):
        Wp = Wm * g1[:, None]
        bp = np.asarray(inputs[bm], f32) + beta1 @ Wm
        return Wp, bp

    Wqp, bqp = fold(Wq, "bq")
    Wkp, bkp = fold(Wk, "bk")
    Wvp, bvp = fold(Wv, "bv")
    W1p = W1 * g2[:, None]
    b1p = np.asarray(inputs["b1"], f32) + beta2 @ W1
    b2p = np.asarray(inputs["b2"], f32) + beta2

    bf = ml_dtypes.bfloat16
    f8 = ml_dtypes.float8_e4m3

    def wtile(Wm, ntile, dtype):
        return np.ascontiguousarray(
            Wm.reshape(ntile, P, Wm.shape[1]).transpose(1, 0, 2)).astype(dtype)

    def wtile_split(Wm, ntile, scale):
        """fp8 value + residual of scale*Wm, packed [P, ntile, 2, cols]."""
        ws = (scale * Wm).astype(f32)
        a = np.clip(ws, -240, 240).astype(f8)
        r = np.clip(ws - a.astype(f32), -240, 240).astype(f8)
        at = wtile(a.astype(f32), ntile, f8)
        rt = wtile(r.astype(f32), ntile, f8)
        return np.ascontiguousarray(np.stack([at, rt], axis=2))

    def btile(bv_, ntile):
        return np.ascontiguousarray(bv_.reshape(ntile, P).T).astype(f32)

    E = np.zeros((2, DT, P), f32)
    for t in range(DT):
        for m in range(P):
            E[m // HD, t, m] = 1.0
    E = E.astype(bf)

    w2id = np.zeros((P, DT, P), f32)
    for mt in range(DT):
        for p in range(P):
            w2id[p, mt, p] = 2048.0 * g2[mt * P + p]
    w2id = w2id.astype(bf)

    wq_t = wtile(WSC * Wqp, DT, f8)
    wk_t = wtile(WSC * Wkp, DT, f8)
    wv_t = wtile(WSC * Wvp, DT, f8)

    def reorder_qk(wt):
        w = wt.reshape(P, DT, DT, P)
        return np.ascontiguousarray(w.transpose(0, 2, 1, 3))

    def reorder_v(wt):
        w = wt.reshape(P, DT, 2, 512)
        return np.ascontiguousarray(w.transpose(0, 2, 1, 3))

    w1s = wtile_split(W1p, DT, WSC)
    w1s = np.ascontiguousarray(
        w1s.reshape(P, DT, 2, FT, P).transpose(0, 3, 1, 2, 4))
    w2s = wtile_split(W2, FT, 64.0)
    w2s = np.ascontiguousarray(
        w2s.reshape(P, FT, 2, DT, P).transpose(0, 3, 1, 2, 4))

    return {
        "wq": reorder_qk(wq_t), "wk": reorder_qk(wk_t),
        "wv": reorder_v(wv_t),
        "w1": w1s, "w2a": w2s, "w2id": w2id,
        "bq": btile(WSC * bqp, DT), "bk": btile(WSC * bkp, DT),
        "bvp": bvp,
        "b1": btile(WSC * b1p, FT), "b2": btile(b2p, DT),
        "emat": E,
    }


def _per_core_inputs(inputs, shared):
    x = np.asarray(inputs["x"], np.float64)
    f8 = ml_dtypes.float8_e4m3
    bf = ml_dtypes.bfloat16
    # exact LN1 on host (g1/beta1 are folded into the projection weights)
    mu = x.mean(-1, keepdims=True)
    var = ((x - mu) ** 2).mean(-1, keepdims=True)
    h = ((x - mu) / np.sqrt(var + EPS)).astype(np.float32)
    xf = np.asarray(inputs["x"], np.float32)
    maps = []
    for c in range(NCORES):
        b, hf = c // 2, c % 2
        hTn = h[b].T.reshape(DT, P, S).transpose(1, 0, 2)
        if hf == 1:
            hTn = np.concatenate([hTn[:, :, SQ:], hTn[:, :, :SQ]], axis=2)
        hTn = np.ascontiguousarray(hTn)
        xTn = xf[b].T.reshape(DT, P, S).transpose(1, 0, 2)
        xown = xTn[:, :, hf * SQ:(hf + 1) * SQ]
        # fold the V bias into the residual: attn(v + bv) = attn(v) + bv
        xown = xown + shared["bvp"].reshape(DT, P).transpose(1, 0)[:, :, None]
        m = {k: v for k, v in shared.items() if k != "bvp"}
        m["h8d"] = np.clip(hTn, -240, 240).astype(f8)
        m["xqd"] = np.ascontiguousarray(xown).astype(bf)
        maps.append(m)
    return maps


def _get_sharded():
    """Build (once) the nc + jitted shard_map executable."""
    if "sharded" in _CACHE:
        return _CACHE["sharded"]

    import jax
    from jax.sharding import Mesh, PartitionSpec
    from jax.experimental.shard_map import shard_map
    from concourse import bass2jax
    from concourse import mybir as _mybir

    bass2jax.install_neuronx_cc_hook()
    nc = _build_nc()

    partition_name = (nc.partition_id_tensor.name
                      if nc.partition_id_tensor else None)
    in_names, out_names, out_avals, zero_shapes = [], [], [], []
    for alloc in nc.m.functions[0].allocations:
        if not isinstance(alloc, _mybir.MemoryLocationSet):
            continue
        name = alloc.memorylocations[0].name
        if alloc.kind == "ExternalInput":
            if name != partition_name:
                in_names.append(name)
        elif alloc.kind == "ExternalOutput":
            shape = tuple(alloc.tensor_shape)
            dtype = _mybir.dt.np(alloc.dtype)
            out_names.append(name)
            out_avals.append(jax.core.ShapedArray(shape, dtype))
            zero_shapes.append((shape, dtype))
    n_params = len(in_names)
    all_names = in_names + out_names
    if partition_name is not None:
        all_names = all_names + [partition_name]
    donate = tuple(range(n_params, n_params + len(out_names)))

    def _body(*args):
        operands = list(args)
        if partition_name is not None:
            operands.append(bass2jax.partition_id_tensor())
        outs = bass2jax._bass_exec_p.bind(
            *operands,
            out_avals=tuple(out_avals),
            in_names=tuple(all_names),
            out_names=tuple(out_names),
            lowering_input_output_aliases=(),
            sim_require_finite=True,
            sim_require_nnan=True,
            nc=nc,
        )
        return tuple(outs)

    devices = jax.devices()[:NCORES]
    mesh = Mesh(np.asarray(devices), ("core",))
    nin = n_params + len(out_names)
    sharded = jax.jit(
        shard_map(_body, mesh=mesh,
                  in_specs=(PartitionSpec("core"),) * nin,
                  out_specs=(PartitionSpec("core"),) * len(out_names),
                  check_rep=False),
        donate_argnums=donate, keep_unused=True)

    _CACHE["sharded"] = (nc, sharded, in_names, out_names, out_avals,
                         zero_shapes)
    return _CACHE["sharded"]


def _concat_inputs(in_maps):
    _, _, in_names, _, _, zero_shapes = _get_sharded()
    concat_in = [
        np.concatenate([np.asarray(in_maps[c][n]) for c in range(NCORES)],
                       axis=0)
        for n in in_names
    ]
    concat_zeros = [
        np.zeros((NCORES * s[0], *s[1:]), d) for (s, d) in zero_shapes
    ]
    return concat_in, concat_zeros


def _run(in_maps):
    nc, fn, in_names, out_names, out_avals, zero_shapes = _get_sharded()
    concat_in, concat_zeros = _concat_inputs(in_maps)
    outs = fn(*concat_in, *concat_zeros)
    res = []
    for c in range(NCORES):
        res.append({
            name: np.asarray(outs[i]).reshape(NCORES, *out_avals[i].shape)[c]
            for i, name in enumerate(out_names)
        })
    return res


def kernel(**inputs):
    shared = _prep_shared(inputs)
    in_maps = _per_core_inputs(inputs, shared)
    res = _run(in_maps)
    out = np.empty((B, S, D), np.float32)
    for c in range(NCORES):
        b, hf = c // 2, c % 2
        o = res[c]["OUT"]                       # [P, DT, SQ]
        out[b, hf * SQ:(hf + 1) * SQ, :] = o.transpose(2, 1, 0).reshape(SQ, D)
    return out
